# revision 61
# baseline (speedup 1.0000x reference)
"""Trainium2 Bass kernel: per-(image, channel) class-mean replacement (segment mean + gather).

Input:  img [8, 128, 256, 256] f32, gt [8, 1, 256, 256] int32 (labels in [0, 21))
Output: out[b, c, h, w] = mean over pixels p of img[b, c, p] where gt[b, p] == gt[b, h, w]

Sharding: data-parallel over batch — each of the 8 NeuronCores processes one image.

Per-core algorithm (C=128 channels on partitions, HW=65536 pixels on free axis):
  Setup:    gt -> chunk-major gtT [128pix, 512chunk] via PE transposes;
            class-major one-hot planes stash[p, c*512+gc] = (gtT[p,gc]==c),
            built in 64-chunk spans: span 0 on DVE (gates the first sums
            matmul); spans 1-7 split DVE (in-loop) / Pool (upfront).
  Phase 1:  PE-transpose img chunks; copy PSUM->SBUF with
            ->bf16 cast (DVE/Act alternating); sums matmul SWAPPED: stationary =
            imgT chunk [128px,128ch], moving = one-hot view [128px,21cls] ->
            accumulate sumsT[128ch,21cls] in PSUM (21-col outputs are nearly
            free). Counts via lhsT=onehot, rhs=ones -> cnt[21,1]. Sums matmuls
            issue two 512-px groups late (software pipelining) so the in-order
            PE queue never blocks on the copies. Phase-2 ohT pre-transposes for
            the first PRE_G groups are interleaved (Pool joins the copy
            rotation only after its span work is done).
  Means:    sumsT -> SBUF -> PE-transpose -> meansT[21,128] bf16 = sums*rcp(cnt).
  Phase 2:  out[128ch,512px] = meansT^T @ ohT[21,512] per group; copy PSUM->SBUF
            as bf16; DMA out 2048-px tiles. Output DRAM tensor is bf16 (host
            casts back to f32) — halves write bandwidth at zero added error
            since means are already bf16. Pre-transposed and JIT output tiles
            alternate 1:1; JIT ohT pairs are transposed two tiles ahead.
            NOTE: GPSIMD/Pool must never read PSUM (walrus rejects it), so all
            PSUM->SBUF copies stay on DVE/Act.
"""

import os
import sys

for _p in ("/opt/trn_rl_repo", "/root/.axon_site/_ro/trn_rl_repo"):
    if os.path.isdir(_p) and _p not in sys.path:
        sys.path.append(_p)

import numpy as np

P = 128          # channels == SBUF partitions
HW = 256 * 256   # pixels per image
NCLS = 21
CH = 128         # pixels per matmul chunk
NCH = HW // CH   # 512 chunks
FB = 2048        # pixels per DMA tile
NB = HW // FB    # 32 big tiles
NGR = HW // 512  # 128 phase-2 groups (512 px each)
PRE_G = 92       # groups whose ohT is pre-transposed during phase 1
EPS = 1e-8
N_CORES = 8

_CACHE = {}


def _build_module():
    import concourse.bacc as bacc
    import concourse.mybir as mybir
    import concourse.tile as tile
    from concourse.masks import make_identity

    f32 = mybir.dt.float32
    bf16 = mybir.dt.bfloat16
    i32 = mybir.dt.int32
    EQ = mybir.AluOpType.is_equal
    MULT = mybir.AluOpType.mult

    nc = bacc.Bacc("TRN2", target_bir_lowering=False, debug=False)
    img = nc.dram_tensor("img", [P, HW], f32, kind="ExternalInput")
    gt = nc.dram_tensor("gt", [HW], i32, kind="ExternalInput")
    out = nc.dram_tensor("out", [P, HW], bf16, kind="ExternalOutput")

    with tile.TileContext(nc) as tc:
        with (
            tc.tile_pool(name="constp", bufs=1) as constp,
            tc.tile_pool(name="imgp", bufs=5) as imgp,
            tc.tile_pool(name="rhsp", bufs=10) as rhsp,
            tc.tile_pool(name="ohsbp", bufs=2) as ohsbp,
            tc.tile_pool(name="outp", bufs=5) as outp,
            tc.tile_pool(name="psA", bufs=4, space="PSUM") as psA,
            tc.tile_pool(name="psB", bufs=1, space="PSUM") as psB,
            tc.tile_pool(name="psC", bufs=2, space="PSUM") as psC,
        ):
            # ---- constants ----
            ident32 = constp.tile([P, P], f32, tag="id32")
            make_identity(nc, ident32[:])
            ident16 = constp.tile([P, P], bf16, tag="id16")
            nc.vector.tensor_copy(out=ident16[:], in_=ident32[:])
            ones1 = constp.tile([P, 1], bf16, tag="ones1")
            nc.vector.memset(ones1[:], 1.0)

            # gt: load [32, 2048], cast f32 (Act), PE-transpose 16 blocks into
            # chunk-major gtT (block b holds chunks {16r+b}; stride-16 dest AP).
            # gt staging borrows imgp slots (same per-partition footprint).
            gtn_i = imgp.tile([32, HW // 32], i32, tag="img")
            gtn = imgp.tile([32, HW // 32], f32, tag="img")
            # gt loads FIRST on the SP queue (ahead of the img stream on the
            # serial DMA engines) in 2 pieces, casts pipelined on Act
            gt_pc = (HW // 32) // 2
            for pc in range(2):
                nc.sync.dma_start(
                    out=gtn_i[:, pc * gt_pc : (pc + 1) * gt_pc],
                    in_=gt.ap().rearrange("(p f) -> p f", p=32)[
                        :, pc * gt_pc : (pc + 1) * gt_pc
                    ],
                )
            for pc in range(2):
                nc.scalar.copy(
                    out=gtn[:, pc * gt_pc : (pc + 1) * gt_pc],
                    in_=gtn_i[:, pc * gt_pc : (pc + 1) * gt_pc],
                )
            gtT = constp.tile([P, NCH], f32, tag="gtT")
            gtTv = gtT[:].rearrange("p (r b) -> p r b", b=16)
            # all 16 [32,128]->[128,32] block transposes land in ONE psA tile,
            # then a single strided copy scatters them into chunk-major gtT —
            # avoids 16 cross-engine sem round-trips through a 2-deep pool
            gps16 = psA.tile([P, 512], f32, tag="a")
            for b in range(16):
                nc.tensor.transpose(
                    out=gps16[:, b * 32 : (b + 1) * 32],
                    in_=gtn[:, b * P : (b + 1) * P],
                    identity=ident32[0:32, 0:32],
                )
            nc.vector.tensor_copy(
                out=gtTv[:, :, :],
                in_=gps16[:].rearrange("p (b r) -> p r b", b=16),
            )

            # class-major one-hot planes: stash[p, c*NCH + gc] = (gtT[p,gc]==c)
            stash = constp.tile([P, NCLS * NCH], bf16, tag="stash")

            def issue_span(s, eng):
                for c in range(NCLS):
                    eng.tensor_scalar(
                        stash[:, c * NCH + 64 * s : c * NCH + 64 * (s + 1)],
                        gtT[:, 64 * s : 64 * (s + 1)],
                        float(c),
                        None,
                        EQ,
                    )

            # span 0 on DVE (it gates the first sums matmul). Spans 1-7 are
            # split by class: Pool halves issue upfront (Pool is idle), DVE
            # halves issue from inside the tile loop so the rhs copies are not
            # queued behind them — span s gates only tiles 4s and later.
            issue_span(0, nc.vector)

            def issue_span_half(s, eng_id):
                for c in range(NCLS):
                    if c % 2 == eng_id:
                        continue
                    eng = nc.vector if c % 2 == 0 else nc.gpsimd
                    eng.tensor_scalar(
                        stash[:, c * NCH + 64 * s : c * NCH + 64 * (s + 1)],
                        gtT[:, 64 * s : 64 * (s + 1)],
                        float(c),
                        None,
                        EQ,
                    )

            for s in range(1, 8):
                issue_span_half(s, 0)  # odd classes -> Pool, upfront
            stashv = stash[:].rearrange("p (c j) -> p c j", c=NCLS)

            def ohview(gc):
                return stashv[:, :, gc]  # [128px, 21cls]

            # pre-transposed ohT for groups [0, PRE_G)
            ohstash = constp.tile([32, PRE_G * 512], bf16, tag="ohstash")

            sums = psB.tile([P, NCLS], f32, tag="sums")
            cnt = psB.tile([NCLS, 1], f32, tag="cnt")

            def copy_by(eng, dst, src):
                if eng == 0:
                    nc.vector.tensor_copy(out=dst, in_=src)
                elif eng == 1:
                    nc.scalar.copy(out=dst, in_=src)
                else:
                    nc.gpsimd.tensor_copy(out=dst, in_=src)

            def pre_transpose_group(g, eng):
                ohps = psC.tile([32, 512], bf16, tag="c")
                for q in range(4):
                    nc.tensor.transpose(
                        out=ohps[0:NCLS, q * CH : (q + 1) * CH],
                        in_=ohview(g * 4 + q),
                        identity=ident16[:],
                    )
                copy_by(eng, ohstash[0:NCLS, g * 512 : (g + 1) * 512], ohps[0:NCLS, :])

            # ---- phase 1 ----
            LAG = 2
            pre_done = 0
            pending = []

            def issue_sums(g4, rhs4):
                for q in range(4):
                    gc = g4 * 4 + q
                    nc.tensor.matmul(
                        out=sums[:],
                        lhsT=rhs4[:, q * CH : (q + 1) * CH],
                        rhs=ohview(gc),
                        start=(gc == 0),
                        stop=(gc == NCH - 1),
                    )
                    nc.tensor.matmul(
                        out=cnt[:],
                        lhsT=ohview(gc),
                        rhs=ones1[:],
                        start=(gc == 0),
                        stop=(gc == NCH - 1),
                    )

            pre_done = 0
            for t in range(NB):
                ib = imgp.tile([P, FB], f32, tag="img")
                for h in range(2):
                    nc.sync.dma_start(
                        out=ib[:, h * 1024 : (h + 1) * 1024],
                        in_=img.ap()[:, t * FB + h * 1024 : t * FB + (h + 1) * 1024],
                    )
                for jj in range(4):
                    g4 = t * 4 + jj
                    tp4 = psA.tile([P, 512], f32, tag="a")
                    for q in range(4):
                        nc.tensor.transpose(
                            out=tp4[:, q * CH : (q + 1) * CH],
                            in_=ib[:, (jj * 4 + q) * CH : (jj * 4 + q + 1) * CH],
                            identity=ident32[:],
                        )
                    rhs4 = rhsp.tile([P, 512], bf16, tag="rhs")
                    copy_by(g4 % 2, rhs4[:], tp4[:])
                    pending.append((g4, rhs4))
                    if len(pending) > LAG:
                        issue_sums(*pending.pop(0))
                # DVE half of span t//4+1 right after this tile's copies
                if t % 4 == 0 and t // 4 + 1 < 8:
                    issue_span_half(t // 4 + 1, 1)  # even classes -> DVE
                # two phase-2 ohT pre-transposes per tile; Pool joins the
                # copy rotation only once its span building is long done
                target = min(PRE_G // 2, ((t + 1) * (PRE_G // 2)) // NB)
                while pre_done < target:
                    pre_transpose_group(2 * pre_done, pre_done % 2)
                    pre_transpose_group(2 * pre_done + 1, (pre_done + 1) % 2)
                    pre_done += 1
            while pending:
                issue_sums(*pending.pop(0))

            # ---- means: meansT[21,128] bf16 = sumsT^T * 1/(cnt+eps) ----
            cnte = constp.tile([NCLS, 1], f32, tag="cnte")
            nc.vector.tensor_scalar_add(cnte[:], cnt[:], EPS)
            rcp = constp.tile([NCLS, 1], f32, tag="rcp")
            nc.vector.reciprocal(out=rcp[:], in_=cnte[:])
            sms = constp.tile([P, NCLS], f32, tag="sms")
            nc.vector.tensor_copy(out=sms[:], in_=sums[:])
            smsP = psC.tile([NCLS, P], f32, tag="c")
            nc.tensor.transpose(out=smsP[:], in_=sms[:], identity=ident32[:])
            meansT = constp.tile([NCLS, P], bf16, tag="meansT")
            nc.vector.tensor_scalar(meansT[:], smsP[:], rcp[:, 0:1], None, MULT)

            # ---- phase 2: out[128ch, px] = meansT^T @ ohT ----
            # Pre-transposed and JIT output tiles alternate 1:1; each JIT
            # tile's two ohT pairs are transposed+copied one position ahead.
            n_pre_t = PRE_G // 4
            n_jit = NB - n_pre_t
            tile_order, pi, ji, err = [], 0, n_pre_t, 0
            for k in range(NB):
                err += n_jit
                if err >= NB and ji < NB and pi > 0:
                    tile_order.append(ji)
                    ji += 1
                    err -= NB
                else:
                    tile_order.append(pi)
                    pi += 1
            jit_ohs = {}
            jit_cnt = [0]

            def stage_jit(tt):
                pair = []
                for half in range(2):
                    ohps2 = psC.tile([32, 1024], bf16, tag="c")
                    for qq in range(8):
                        nc.tensor.transpose(
                            out=ohps2[0:NCLS, qq * CH : (qq + 1) * CH],
                            in_=ohview((4 * tt + 2 * half) * 4 + qq),
                            identity=ident16[:],
                        )
                    ohs = ohsbp.tile([32, 1024], bf16, tag="oh")
                    copy_by(jit_cnt[0] % 2, ohs[0:NCLS, :], ohps2[0:NCLS, :])
                    jit_cnt[0] += 1
                    pair.append(ohs)
                jit_ohs[tt] = pair

            # stage each JIT tile's ohT pairs two positions ahead
            if len(tile_order) > 1 and tile_order[1] >= n_pre_t:
                stage_jit(tile_order[1])
            for pos, tt in enumerate(tile_order):
                if pos + 2 < len(tile_order) and tile_order[pos + 2] >= n_pre_t:
                    stage_jit(tile_order[pos + 2])
                jit = tt >= n_pre_t
                ob4 = outp.tile([P, FB], bf16, tag="ob")
                for k in range(4):
                    g = 4 * tt + k
                    if jit:
                        rhs_ap = jit_ohs[tt][k // 2][0:NCLS, (k % 2) * 512 : (k % 2 + 1) * 512]
                    else:
                        rhs_ap = ohstash[0:NCLS, g * 512 : (g + 1) * 512]
                    op_ = psA.tile([P, 512], f32, tag="a")
                    nc.tensor.matmul(
                        out=op_[:], lhsT=meansT[:], rhs=rhs_ap, start=True, stop=True
                    )
                    rot = ((1, 0, 1, 0), (0, 1, 0, 1))[pos % 2]
                    copy_by(rot[k], ob4[:, k * 512 : (k + 1) * 512], op_[:])
                if pos == len(tile_order) - 1:
                    # split first tiles' DMAs (stream starts sooner after
                    # means) and the last tile's (tail drains sooner)
                    for s in range(4):
                        nc.sync.dma_start(
                            out=out.ap()[:, (4 * tt + s) * 512 : (4 * tt + s + 1) * 512],
                            in_=ob4[:, s * 512 : (s + 1) * 512],
                        )
                else:
                    nc.sync.dma_start(
                        out=out.ap()[:, tt * FB : (tt + 1) * FB], in_=ob4[:]
                    )

    nc.compile()
    return nc


def get_module():
    if "nc" not in _CACHE:
        _CACHE["nc"] = _build_module()
    return _CACHE["nc"]


def kernel(img, gt):
    from concourse.bass_utils import run_bass_kernel_spmd

    img = np.asarray(img)
    gt = np.asarray(gt)
    B, C, H, W = img.shape
    assert (B, C, H * W) == (N_CORES, P, HW), (img.shape,)
    img2 = np.ascontiguousarray(img.reshape(B, C, H * W))
    gt2 = np.ascontiguousarray(gt.reshape(B, H * W))

    nc = get_module()
    in_maps = [{"img": img2[i], "gt": gt2[i]} for i in range(B)]
    res = run_bass_kernel_spmd(nc, in_maps, core_ids=list(range(N_CORES)))
    out = np.stack(
        [np.asarray(res.results[i]["out"]).astype(np.float32) for i in range(B)],
        axis=0,
    )
    return out.reshape(B, C, H, W)


if __name__ == "__main__":
    rng = np.random.default_rng(0)
    img = rng.standard_normal((8, 128, 256, 256), dtype=np.float32)
    gt = rng.integers(0, NCLS, size=(8, 1, 256, 256), dtype=np.int32)
    out = kernel(img=img, gt=gt)
    print("out", out.shape, out.dtype)


# revision 62
# speedup vs baseline: 1.0010x; 1.0010x over previous
"""Trainium2 Bass kernel: per-(image, channel) class-mean replacement (segment mean + gather).

Input:  img [8, 128, 256, 256] f32, gt [8, 1, 256, 256] int32 (labels in [0, 21))
Output: out[b, c, h, w] = mean over pixels p of img[b, c, p] where gt[b, p] == gt[b, h, w]

Sharding: data-parallel over batch — each of the 8 NeuronCores processes one image.

Per-core algorithm (C=128 channels on partitions, HW=65536 pixels on free axis):
  Setup:    gt -> chunk-major gtT [128pix, 512chunk] via PE transposes;
            class-major one-hot planes stash[p, c*512+gc] = (gtT[p,gc]==c),
            built in 64-chunk spans: span 0 on DVE (gates the first sums
            matmul); spans 1-7 split DVE (in-loop) / Pool (upfront).
  Phase 1:  PE-transpose img chunks; copy PSUM->SBUF with
            ->bf16 cast (DVE/Act alternating); sums matmul SWAPPED: stationary =
            imgT chunk [128px,128ch], moving = one-hot view [128px,21cls] ->
            accumulate sumsT[128ch,21cls] in PSUM (21-col outputs are nearly
            free). Counts via lhsT=onehot, rhs=ones -> cnt[21,1]. Sums matmuls
            issue two 512-px groups late (software pipelining) so the in-order
            PE queue never blocks on the copies. Phase-2 ohT pre-transposes for
            the first PRE_G groups are interleaved (Pool joins the copy
            rotation only after its span work is done).
  Means:    sumsT -> SBUF -> PE-transpose -> meansT[21,128] bf16 = sums*rcp(cnt).
  Phase 2:  out[128ch,512px] = meansT^T @ ohT[21,512] per group; copy PSUM->SBUF
            as bf16; DMA out 2048-px tiles. Output DRAM tensor is bf16 (host
            casts back to f32) — halves write bandwidth at zero added error
            since means are already bf16. Pre-transposed and JIT output tiles
            alternate 1:1; JIT ohT pairs are transposed two tiles ahead.
            NOTE: GPSIMD/Pool must never read PSUM (walrus rejects it), so all
            PSUM->SBUF copies stay on DVE/Act.
"""

import os
import sys

for _p in ("/opt/trn_rl_repo", "/root/.axon_site/_ro/trn_rl_repo"):
    if os.path.isdir(_p) and _p not in sys.path:
        sys.path.append(_p)

import numpy as np

P = 128          # channels == SBUF partitions
HW = 256 * 256   # pixels per image
NCLS = 21
CH = 128         # pixels per matmul chunk
NCH = HW // CH   # 512 chunks
FB = 2048        # pixels per DMA tile
NB = HW // FB    # 32 big tiles
NGR = HW // 512  # 128 phase-2 groups (512 px each)
PRE_G = 92       # groups whose ohT is pre-transposed during phase 1
EPS = 1e-8
N_CORES = 8

_CACHE = {}


def _build_module():
    import concourse.bacc as bacc
    import concourse.mybir as mybir
    import concourse.tile as tile
    from concourse.masks import make_identity

    f32 = mybir.dt.float32
    bf16 = mybir.dt.bfloat16
    i32 = mybir.dt.int32
    EQ = mybir.AluOpType.is_equal
    MULT = mybir.AluOpType.mult

    nc = bacc.Bacc("TRN2", target_bir_lowering=False, debug=False)
    img = nc.dram_tensor("img", [P, HW], f32, kind="ExternalInput")
    gt = nc.dram_tensor("gt", [HW], i32, kind="ExternalInput")
    out = nc.dram_tensor("out", [P, HW], bf16, kind="ExternalOutput")

    with tile.TileContext(nc) as tc:
        with (
            tc.tile_pool(name="constp", bufs=1) as constp,
            tc.tile_pool(name="imgp", bufs=5) as imgp,
            tc.tile_pool(name="rhsp", bufs=10) as rhsp,
            tc.tile_pool(name="ohsbp", bufs=2) as ohsbp,
            tc.tile_pool(name="outp", bufs=5) as outp,
            tc.tile_pool(name="psA", bufs=4, space="PSUM") as psA,
            tc.tile_pool(name="psB", bufs=1, space="PSUM") as psB,
            tc.tile_pool(name="psC", bufs=2, space="PSUM") as psC,
        ):
            # ---- constants ----
            ident32 = constp.tile([P, P], f32, tag="id32")
            make_identity(nc, ident32[:])
            ident16 = constp.tile([P, P], bf16, tag="id16")
            nc.vector.tensor_copy(out=ident16[:], in_=ident32[:])
            ones1 = constp.tile([P, 1], bf16, tag="ones1")
            nc.vector.memset(ones1[:], 1.0)

            # gt: load [32, 2048], cast f32 (Act), PE-transpose 16 blocks into
            # chunk-major gtT (block b holds chunks {16r+b}; stride-16 dest AP).
            # gt staging borrows imgp slots (same per-partition footprint).
            gtn_i = imgp.tile([32, HW // 32], i32, tag="img")
            gtn = imgp.tile([32, HW // 32], f32, tag="img")
            # gt loads FIRST on the SP queue (ahead of the img stream on the
            # serial DMA engines) in 2 pieces, casts pipelined on Act
            gt_pc = (HW // 32) // 2
            for pc in range(2):
                nc.sync.dma_start(
                    out=gtn_i[:, pc * gt_pc : (pc + 1) * gt_pc],
                    in_=gt.ap().rearrange("(p f) -> p f", p=32)[
                        :, pc * gt_pc : (pc + 1) * gt_pc
                    ],
                )
            for pc in range(2):
                nc.scalar.copy(
                    out=gtn[:, pc * gt_pc : (pc + 1) * gt_pc],
                    in_=gtn_i[:, pc * gt_pc : (pc + 1) * gt_pc],
                )
            gtT = constp.tile([P, NCH], f32, tag="gtT")
            gtTv = gtT[:].rearrange("p (r b) -> p r b", b=16)
            # all 16 [32,128]->[128,32] block transposes land in ONE psA tile,
            # then a single strided copy scatters them into chunk-major gtT —
            # avoids 16 cross-engine sem round-trips through a 2-deep pool
            gps16 = psA.tile([P, 512], f32, tag="a")
            for b in range(16):
                nc.tensor.transpose(
                    out=gps16[:, b * 32 : (b + 1) * 32],
                    in_=gtn[:, b * P : (b + 1) * P],
                    identity=ident32[0:32, 0:32],
                )
            nc.vector.tensor_copy(
                out=gtTv[:, :, :],
                in_=gps16[:].rearrange("p (b r) -> p r b", b=16),
            )

            # class-major one-hot planes: stash[p, c*NCH + gc] = (gtT[p,gc]==c)
            stash = constp.tile([P, NCLS * NCH], bf16, tag="stash")

            def issue_span(s, eng):
                for c in range(NCLS):
                    eng.tensor_scalar(
                        stash[:, c * NCH + 64 * s : c * NCH + 64 * (s + 1)],
                        gtT[:, 64 * s : 64 * (s + 1)],
                        float(c),
                        None,
                        EQ,
                    )

            # span 0 on DVE (it gates the first sums matmul). Spans 1-7 are
            # split by class: Pool halves issue upfront (Pool is idle), DVE
            # halves issue from inside the tile loop so the rhs copies are not
            # queued behind them — span s gates only tiles 4s and later.
            issue_span(0, nc.vector)

            def issue_span_half(s, eng_id):
                for c in range(NCLS):
                    if c % 2 == eng_id:
                        continue
                    eng = nc.vector if c % 2 == 0 else nc.gpsimd
                    eng.tensor_scalar(
                        stash[:, c * NCH + 64 * s : c * NCH + 64 * (s + 1)],
                        gtT[:, 64 * s : 64 * (s + 1)],
                        float(c),
                        None,
                        EQ,
                    )

            for s in range(1, 8):
                issue_span_half(s, 0)  # odd classes -> Pool, upfront
            stashv = stash[:].rearrange("p (c j) -> p c j", c=NCLS)

            def ohview(gc):
                return stashv[:, :, gc]  # [128px, 21cls]

            # pre-transposed ohT for groups [0, PRE_G)
            ohstash = constp.tile([32, PRE_G * 512], bf16, tag="ohstash")

            sums = psB.tile([P, NCLS], f32, tag="sums")
            cnt = psB.tile([NCLS, 1], f32, tag="cnt")

            def copy_by(eng, dst, src):
                if eng == 0:
                    nc.vector.tensor_copy(out=dst, in_=src)
                elif eng == 1:
                    nc.scalar.copy(out=dst, in_=src)
                else:
                    nc.gpsimd.tensor_copy(out=dst, in_=src)

            def pre_transpose_group(g, eng):
                ohps = psC.tile([32, 512], bf16, tag="c")
                for q in range(4):
                    nc.tensor.transpose(
                        out=ohps[0:NCLS, q * CH : (q + 1) * CH],
                        in_=ohview(g * 4 + q),
                        identity=ident16[:],
                    )
                copy_by(eng, ohstash[0:NCLS, g * 512 : (g + 1) * 512], ohps[0:NCLS, :])

            # ---- phase 1 ----
            LAG = 2
            pre_done = 0
            pending = []

            def issue_sums(g4, rhs4):
                for q in range(4):
                    gc = g4 * 4 + q
                    nc.tensor.matmul(
                        out=sums[:],
                        lhsT=rhs4[:, q * CH : (q + 1) * CH],
                        rhs=ohview(gc),
                        start=(gc == 0),
                        stop=(gc == NCH - 1),
                    )
                    nc.tensor.matmul(
                        out=cnt[:],
                        lhsT=ohview(gc),
                        rhs=ones1[:],
                        start=(gc == 0),
                        stop=(gc == NCH - 1),
                    )

            pre_done = 0
            for t in range(NB):
                ib = imgp.tile([P, FB], f32, tag="img")
                for h in range(2):
                    nc.sync.dma_start(
                        out=ib[:, h * 1024 : (h + 1) * 1024],
                        in_=img.ap()[:, t * FB + h * 1024 : t * FB + (h + 1) * 1024],
                    )
                for jj in range(4):
                    g4 = t * 4 + jj
                    tp4 = psA.tile([P, 512], f32, tag="a")
                    for q in range(4):
                        nc.tensor.transpose(
                            out=tp4[:, q * CH : (q + 1) * CH],
                            in_=ib[:, (jj * 4 + q) * CH : (jj * 4 + q + 1) * CH],
                            identity=ident32[:],
                        )
                    rhs4 = rhsp.tile([P, 512], bf16, tag="rhs")
                    copy_by(g4 % 2, rhs4[:], tp4[:])
                    pending.append((g4, rhs4))
                    if len(pending) > LAG:
                        issue_sums(*pending.pop(0))
                # DVE half of span t//4+1 right after this tile's copies
                if t % 4 == 0 and t // 4 + 1 < 8:
                    issue_span_half(t // 4 + 1, 1)  # even classes -> DVE
                # two phase-2 ohT pre-transposes per tile; Pool joins the
                # copy rotation only once its span building is long done
                target = min(PRE_G // 2, ((t + 1) * (PRE_G // 2)) // NB)
                while pre_done < target:
                    pre_transpose_group(2 * pre_done, pre_done % 2)
                    pre_transpose_group(2 * pre_done + 1, (pre_done + 1) % 2)
                    pre_done += 1
            while pending:
                issue_sums(*pending.pop(0))

            # ---- means: meansT[21,128] bf16 = sumsT^T * 1/(cnt+eps) ----
            cnte = constp.tile([NCLS, 1], f32, tag="cnte")
            nc.vector.tensor_scalar_add(cnte[:], cnt[:], EPS)
            rcp = constp.tile([NCLS, 1], f32, tag="rcp")
            nc.vector.reciprocal(out=rcp[:], in_=cnte[:])
            sms = constp.tile([P, NCLS], f32, tag="sms")
            nc.vector.tensor_copy(out=sms[:], in_=sums[:])
            smsP = psC.tile([NCLS, P], f32, tag="c")
            nc.tensor.transpose(out=smsP[:], in_=sms[:], identity=ident32[:])
            meansT = constp.tile([NCLS, P], bf16, tag="meansT")
            nc.vector.tensor_scalar(meansT[:], smsP[:], rcp[:, 0:1], None, MULT)

            # ---- phase 2: out[128ch, px] = meansT^T @ ohT ----
            # Pre-transposed and JIT output tiles alternate 1:1; each JIT
            # tile's two ohT pairs are transposed+copied one position ahead.
            n_pre_t = PRE_G // 4
            n_jit = NB - n_pre_t
            tile_order, pi, ji, err = [], 0, n_pre_t, 0
            for k in range(NB):
                err += n_jit
                if err >= NB and ji < NB and pi > 0:
                    tile_order.append(ji)
                    ji += 1
                    err -= NB
                else:
                    tile_order.append(pi)
                    pi += 1
            jit_ohs = {}
            jit_cnt = [0]

            def stage_jit(tt):
                pair = []
                for half in range(2):
                    ohps2 = psC.tile([32, 1024], bf16, tag="c")
                    for qq in range(8):
                        nc.tensor.transpose(
                            out=ohps2[0:NCLS, qq * CH : (qq + 1) * CH],
                            in_=ohview((4 * tt + 2 * half) * 4 + qq),
                            identity=ident16[:],
                        )
                    ohs = ohsbp.tile([32, 1024], bf16, tag="oh")
                    copy_by(jit_cnt[0] % 2, ohs[0:NCLS, :], ohps2[0:NCLS, :])
                    jit_cnt[0] += 1
                    pair.append(ohs)
                jit_ohs[tt] = pair

            # stage each JIT tile's ohT pairs two positions ahead
            if len(tile_order) > 1 and tile_order[1] >= n_pre_t:
                stage_jit(tile_order[1])
            for pos, tt in enumerate(tile_order):
                if pos + 2 < len(tile_order) and tile_order[pos + 2] >= n_pre_t:
                    stage_jit(tile_order[pos + 2])
                jit = tt >= n_pre_t
                ob4 = outp.tile([P, FB], bf16, tag="ob")
                for k in range(4):
                    g = 4 * tt + k
                    if jit:
                        rhs_ap = jit_ohs[tt][k // 2][0:NCLS, (k % 2) * 512 : (k % 2 + 1) * 512]
                    else:
                        rhs_ap = ohstash[0:NCLS, g * 512 : (g + 1) * 512]
                    op_ = psA.tile([P, 512], f32, tag="a")
                    nc.tensor.matmul(
                        out=op_[:], lhsT=meansT[:], rhs=rhs_ap, start=True, stop=True
                    )
                    rot = ((1, 0, 1, 0), (0, 1, 0, 1))[pos % 2]
                    copy_by(rot[k], ob4[:, k * 512 : (k + 1) * 512], op_[:])
                if pos == 0 or pos == len(tile_order) - 1:
                    # split first tiles' DMAs (stream starts sooner after
                    # means) and the last tile's (tail drains sooner)
                    for s in range(4):
                        nc.sync.dma_start(
                            out=out.ap()[:, (4 * tt + s) * 512 : (4 * tt + s + 1) * 512],
                            in_=ob4[:, s * 512 : (s + 1) * 512],
                        )
                else:
                    nc.sync.dma_start(
                        out=out.ap()[:, tt * FB : (tt + 1) * FB], in_=ob4[:]
                    )

    nc.compile()
    return nc


def get_module():
    if "nc" not in _CACHE:
        _CACHE["nc"] = _build_module()
    return _CACHE["nc"]


def kernel(img, gt):
    from concourse.bass_utils import run_bass_kernel_spmd

    img = np.asarray(img)
    gt = np.asarray(gt)
    B, C, H, W = img.shape
    assert (B, C, H * W) == (N_CORES, P, HW), (img.shape,)
    img2 = np.ascontiguousarray(img.reshape(B, C, H * W))
    gt2 = np.ascontiguousarray(gt.reshape(B, H * W))

    nc = get_module()
    in_maps = [{"img": img2[i], "gt": gt2[i]} for i in range(B)]
    res = run_bass_kernel_spmd(nc, in_maps, core_ids=list(range(N_CORES)))
    out = np.stack(
        [np.asarray(res.results[i]["out"]).astype(np.float32) for i in range(B)],
        axis=0,
    )
    return out.reshape(B, C, H, W)


if __name__ == "__main__":
    rng = np.random.default_rng(0)
    img = rng.standard_normal((8, 128, 256, 256), dtype=np.float32)
    gt = rng.integers(0, NCLS, size=(8, 1, 256, 256), dtype=np.int32)
    out = kernel(img=img, gt=gt)
    print("out", out.shape, out.dtype)


# revision 63
# speedup vs baseline: 1.0012x; 1.0003x over previous
"""Trainium2 Bass kernel: per-(image, channel) class-mean replacement (segment mean + gather).

Input:  img [8, 128, 256, 256] f32, gt [8, 1, 256, 256] int32 (labels in [0, 21))
Output: out[b, c, h, w] = mean over pixels p of img[b, c, p] where gt[b, p] == gt[b, h, w]

Sharding: data-parallel over batch — each of the 8 NeuronCores processes one image.

Per-core algorithm (C=128 channels on partitions, HW=65536 pixels on free axis):
  Setup:    gt -> chunk-major gtT [128pix, 512chunk] via PE transposes;
            class-major one-hot planes stash[p, c*512+gc] = (gtT[p,gc]==c),
            built in 64-chunk spans: span 0 on DVE (gates the first sums
            matmul); spans 1-7 split DVE (in-loop) / Pool (upfront).
  Phase 1:  PE-transpose img chunks; copy PSUM->SBUF with
            ->bf16 cast (DVE/Act alternating); sums matmul SWAPPED: stationary =
            imgT chunk [128px,128ch], moving = one-hot view [128px,21cls] ->
            accumulate sumsT[128ch,21cls] in PSUM (21-col outputs are nearly
            free). Counts via lhsT=onehot, rhs=ones -> cnt[21,1]. Sums matmuls
            issue two 512-px groups late (software pipelining) so the in-order
            PE queue never blocks on the copies. Phase-2 ohT pre-transposes for
            the first PRE_G groups are interleaved (Pool joins the copy
            rotation only after its span work is done).
  Means:    sumsT -> SBUF -> PE-transpose -> meansT[21,128] bf16 = sums*rcp(cnt).
  Phase 2:  out[128ch,512px] = meansT^T @ ohT[21,512] per group; copy PSUM->SBUF
            as bf16; DMA out 2048-px tiles. Output DRAM tensor is bf16 (host
            casts back to f32) — halves write bandwidth at zero added error
            since means are already bf16. Pre-transposed and JIT output tiles
            alternate 1:1; JIT ohT pairs are transposed two tiles ahead.
            NOTE: GPSIMD/Pool must never read PSUM (walrus rejects it), so all
            PSUM->SBUF copies stay on DVE/Act.
"""

import os
import sys

for _p in ("/opt/trn_rl_repo", "/root/.axon_site/_ro/trn_rl_repo"):
    if os.path.isdir(_p) and _p not in sys.path:
        sys.path.append(_p)

import numpy as np

P = 128          # channels == SBUF partitions
HW = 256 * 256   # pixels per image
NCLS = 21
CH = 128         # pixels per matmul chunk
NCH = HW // CH   # 512 chunks
FB = 2048        # pixels per DMA tile
NB = HW // FB    # 32 big tiles
NGR = HW // 512  # 128 phase-2 groups (512 px each)
PRE_G = 92       # groups whose ohT is pre-transposed during phase 1
EPS = 1e-8
N_CORES = 8

_CACHE = {}


def _build_module():
    import concourse.bacc as bacc
    import concourse.mybir as mybir
    import concourse.tile as tile
    from concourse.masks import make_identity

    f32 = mybir.dt.float32
    bf16 = mybir.dt.bfloat16
    i32 = mybir.dt.int32
    EQ = mybir.AluOpType.is_equal
    MULT = mybir.AluOpType.mult

    nc = bacc.Bacc("TRN2", target_bir_lowering=False, debug=False)
    img = nc.dram_tensor("img", [P, HW], f32, kind="ExternalInput")
    gt = nc.dram_tensor("gt", [HW], i32, kind="ExternalInput")
    out = nc.dram_tensor("out", [P, HW], bf16, kind="ExternalOutput")

    with tile.TileContext(nc) as tc:
        with (
            tc.tile_pool(name="constp", bufs=1) as constp,
            tc.tile_pool(name="imgp", bufs=6) as imgp,
            tc.tile_pool(name="rhsp", bufs=12) as rhsp,
            tc.tile_pool(name="ohsbp", bufs=2) as ohsbp,
            tc.tile_pool(name="outp", bufs=5) as outp,
            tc.tile_pool(name="psA", bufs=4, space="PSUM") as psA,
            tc.tile_pool(name="psB", bufs=1, space="PSUM") as psB,
            tc.tile_pool(name="psC", bufs=2, space="PSUM") as psC,
        ):
            # ---- constants ----
            ident32 = constp.tile([P, P], f32, tag="id32")
            make_identity(nc, ident32[:])
            ident16 = constp.tile([P, P], bf16, tag="id16")
            nc.vector.tensor_copy(out=ident16[:], in_=ident32[:])
            ones1 = constp.tile([P, 1], bf16, tag="ones1")
            nc.vector.memset(ones1[:], 1.0)

            # gt: load [32, 2048], cast f32 (Act), PE-transpose 16 blocks into
            # chunk-major gtT (block b holds chunks {16r+b}; stride-16 dest AP).
            # gt staging borrows imgp slots (same per-partition footprint).
            gtn_i = imgp.tile([32, HW // 32], i32, tag="img")
            gtn = imgp.tile([32, HW // 32], f32, tag="img")
            # gt loads FIRST on the SP queue (ahead of the img stream on the
            # serial DMA engines) in 2 pieces, casts pipelined on Act
            gt_pc = (HW // 32) // 2
            for pc in range(2):
                nc.sync.dma_start(
                    out=gtn_i[:, pc * gt_pc : (pc + 1) * gt_pc],
                    in_=gt.ap().rearrange("(p f) -> p f", p=32)[
                        :, pc * gt_pc : (pc + 1) * gt_pc
                    ],
                )
            for pc in range(2):
                nc.scalar.copy(
                    out=gtn[:, pc * gt_pc : (pc + 1) * gt_pc],
                    in_=gtn_i[:, pc * gt_pc : (pc + 1) * gt_pc],
                )
            gtT = constp.tile([P, NCH], f32, tag="gtT")
            gtTv = gtT[:].rearrange("p (r b) -> p r b", b=16)
            # all 16 [32,128]->[128,32] block transposes land in ONE psA tile,
            # then a single strided copy scatters them into chunk-major gtT —
            # avoids 16 cross-engine sem round-trips through a 2-deep pool
            gps16 = psA.tile([P, 512], f32, tag="a")
            for b in range(16):
                nc.tensor.transpose(
                    out=gps16[:, b * 32 : (b + 1) * 32],
                    in_=gtn[:, b * P : (b + 1) * P],
                    identity=ident32[0:32, 0:32],
                )
            nc.vector.tensor_copy(
                out=gtTv[:, :, :],
                in_=gps16[:].rearrange("p (b r) -> p r b", b=16),
            )

            # class-major one-hot planes: stash[p, c*NCH + gc] = (gtT[p,gc]==c)
            stash = constp.tile([P, NCLS * NCH], bf16, tag="stash")

            def issue_span(s, eng):
                for c in range(NCLS):
                    eng.tensor_scalar(
                        stash[:, c * NCH + 64 * s : c * NCH + 64 * (s + 1)],
                        gtT[:, 64 * s : 64 * (s + 1)],
                        float(c),
                        None,
                        EQ,
                    )

            # span 0 on DVE (it gates the first sums matmul). Spans 1-7 are
            # split by class: Pool halves issue upfront (Pool is idle), DVE
            # halves issue from inside the tile loop so the rhs copies are not
            # queued behind them — span s gates only tiles 4s and later.
            issue_span(0, nc.vector)

            def issue_span_half(s, eng_id):
                for c in range(NCLS):
                    if c % 2 == eng_id:
                        continue
                    eng = nc.vector if c % 2 == 0 else nc.gpsimd
                    eng.tensor_scalar(
                        stash[:, c * NCH + 64 * s : c * NCH + 64 * (s + 1)],
                        gtT[:, 64 * s : 64 * (s + 1)],
                        float(c),
                        None,
                        EQ,
                    )

            for s in range(1, 8):
                issue_span_half(s, 0)  # odd classes -> Pool, upfront
            stashv = stash[:].rearrange("p (c j) -> p c j", c=NCLS)

            def ohview(gc):
                return stashv[:, :, gc]  # [128px, 21cls]

            # pre-transposed ohT for groups [0, PRE_G)
            ohstash = constp.tile([32, PRE_G * 512], bf16, tag="ohstash")

            sums = psB.tile([P, NCLS], f32, tag="sums")
            cnt = psB.tile([NCLS, 1], f32, tag="cnt")

            def copy_by(eng, dst, src):
                if eng == 0:
                    nc.vector.tensor_copy(out=dst, in_=src)
                elif eng == 1:
                    nc.scalar.copy(out=dst, in_=src)
                else:
                    nc.gpsimd.tensor_copy(out=dst, in_=src)

            def pre_transpose_group(g, eng):
                ohps = psC.tile([32, 512], bf16, tag="c")
                for q in range(4):
                    nc.tensor.transpose(
                        out=ohps[0:NCLS, q * CH : (q + 1) * CH],
                        in_=ohview(g * 4 + q),
                        identity=ident16[:],
                    )
                copy_by(eng, ohstash[0:NCLS, g * 512 : (g + 1) * 512], ohps[0:NCLS, :])

            # ---- phase 1 ----
            LAG = 2
            pre_done = 0
            pending = []

            def issue_sums(g4, rhs4):
                for q in range(4):
                    gc = g4 * 4 + q
                    nc.tensor.matmul(
                        out=sums[:],
                        lhsT=rhs4[:, q * CH : (q + 1) * CH],
                        rhs=ohview(gc),
                        start=(gc == 0),
                        stop=(gc == NCH - 1),
                    )
                    nc.tensor.matmul(
                        out=cnt[:],
                        lhsT=ohview(gc),
                        rhs=ones1[:],
                        start=(gc == 0),
                        stop=(gc == NCH - 1),
                    )

            pre_done = 0
            for t in range(NB):
                ib = imgp.tile([P, FB], f32, tag="img")
                for h in range(2):
                    nc.sync.dma_start(
                        out=ib[:, h * 1024 : (h + 1) * 1024],
                        in_=img.ap()[:, t * FB + h * 1024 : t * FB + (h + 1) * 1024],
                    )
                for jj in range(4):
                    g4 = t * 4 + jj
                    tp4 = psA.tile([P, 512], f32, tag="a")
                    for q in range(4):
                        nc.tensor.transpose(
                            out=tp4[:, q * CH : (q + 1) * CH],
                            in_=ib[:, (jj * 4 + q) * CH : (jj * 4 + q + 1) * CH],
                            identity=ident32[:],
                        )
                    rhs4 = rhsp.tile([P, 512], bf16, tag="rhs")
                    copy_by(g4 % 2, rhs4[:], tp4[:])
                    pending.append((g4, rhs4))
                    if len(pending) > LAG:
                        issue_sums(*pending.pop(0))
                # DVE half of span t//4+1 right after this tile's copies
                if t % 4 == 0 and t // 4 + 1 < 8:
                    issue_span_half(t // 4 + 1, 1)  # even classes -> DVE
                # two phase-2 ohT pre-transposes per tile; Pool joins the
                # copy rotation only once its span building is long done
                target = min(PRE_G // 2, ((t + 1) * (PRE_G // 2)) // NB)
                while pre_done < target:
                    pre_transpose_group(2 * pre_done, pre_done % 2)
                    pre_transpose_group(2 * pre_done + 1, (pre_done + 1) % 2)
                    pre_done += 1
            while pending:
                issue_sums(*pending.pop(0))

            # ---- means: meansT[21,128] bf16 = sumsT^T * 1/(cnt+eps) ----
            cnte = constp.tile([NCLS, 1], f32, tag="cnte")
            nc.vector.tensor_scalar_add(cnte[:], cnt[:], EPS)
            rcp = constp.tile([NCLS, 1], f32, tag="rcp")
            nc.vector.reciprocal(out=rcp[:], in_=cnte[:])
            sms = constp.tile([P, NCLS], f32, tag="sms")
            nc.vector.tensor_copy(out=sms[:], in_=sums[:])
            smsP = psC.tile([NCLS, P], f32, tag="c")
            nc.tensor.transpose(out=smsP[:], in_=sms[:], identity=ident32[:])
            meansT = constp.tile([NCLS, P], bf16, tag="meansT")
            nc.vector.tensor_scalar(meansT[:], smsP[:], rcp[:, 0:1], None, MULT)

            # ---- phase 2: out[128ch, px] = meansT^T @ ohT ----
            # Pre-transposed and JIT output tiles alternate 1:1; each JIT
            # tile's two ohT pairs are transposed+copied one position ahead.
            n_pre_t = PRE_G // 4
            n_jit = NB - n_pre_t
            tile_order, pi, ji, err = [], 0, n_pre_t, 0
            for k in range(NB):
                err += n_jit
                if err >= NB and ji < NB and pi > 0:
                    tile_order.append(ji)
                    ji += 1
                    err -= NB
                else:
                    tile_order.append(pi)
                    pi += 1
            jit_ohs = {}
            jit_cnt = [0]

            def stage_jit(tt):
                pair = []
                for half in range(2):
                    ohps2 = psC.tile([32, 1024], bf16, tag="c")
                    for qq in range(8):
                        nc.tensor.transpose(
                            out=ohps2[0:NCLS, qq * CH : (qq + 1) * CH],
                            in_=ohview((4 * tt + 2 * half) * 4 + qq),
                            identity=ident16[:],
                        )
                    ohs = ohsbp.tile([32, 1024], bf16, tag="oh")
                    copy_by(jit_cnt[0] % 2, ohs[0:NCLS, :], ohps2[0:NCLS, :])
                    jit_cnt[0] += 1
                    pair.append(ohs)
                jit_ohs[tt] = pair

            # stage each JIT tile's ohT pairs two positions ahead
            if len(tile_order) > 1 and tile_order[1] >= n_pre_t:
                stage_jit(tile_order[1])
            for pos, tt in enumerate(tile_order):
                if pos + 2 < len(tile_order) and tile_order[pos + 2] >= n_pre_t:
                    stage_jit(tile_order[pos + 2])
                jit = tt >= n_pre_t
                ob4 = outp.tile([P, FB], bf16, tag="ob")
                for k in range(4):
                    g = 4 * tt + k
                    if jit:
                        rhs_ap = jit_ohs[tt][k // 2][0:NCLS, (k % 2) * 512 : (k % 2 + 1) * 512]
                    else:
                        rhs_ap = ohstash[0:NCLS, g * 512 : (g + 1) * 512]
                    op_ = psA.tile([P, 512], f32, tag="a")
                    nc.tensor.matmul(
                        out=op_[:], lhsT=meansT[:], rhs=rhs_ap, start=True, stop=True
                    )
                    rot = ((1, 0, 1, 0), (0, 1, 0, 1))[pos % 2]
                    copy_by(rot[k], ob4[:, k * 512 : (k + 1) * 512], op_[:])
                if pos == 0 or pos == len(tile_order) - 1:
                    # split first tiles' DMAs (stream starts sooner after
                    # means) and the last tile's (tail drains sooner)
                    for s in range(4):
                        nc.sync.dma_start(
                            out=out.ap()[:, (4 * tt + s) * 512 : (4 * tt + s + 1) * 512],
                            in_=ob4[:, s * 512 : (s + 1) * 512],
                        )
                else:
                    nc.sync.dma_start(
                        out=out.ap()[:, tt * FB : (tt + 1) * FB], in_=ob4[:]
                    )

    nc.compile()
    return nc


def get_module():
    if "nc" not in _CACHE:
        _CACHE["nc"] = _build_module()
    return _CACHE["nc"]


def kernel(img, gt):
    from concourse.bass_utils import run_bass_kernel_spmd

    img = np.asarray(img)
    gt = np.asarray(gt)
    B, C, H, W = img.shape
    assert (B, C, H * W) == (N_CORES, P, HW), (img.shape,)
    img2 = np.ascontiguousarray(img.reshape(B, C, H * W))
    gt2 = np.ascontiguousarray(gt.reshape(B, H * W))

    nc = get_module()
    in_maps = [{"img": img2[i], "gt": gt2[i]} for i in range(B)]
    res = run_bass_kernel_spmd(nc, in_maps, core_ids=list(range(N_CORES)))
    out = np.stack(
        [np.asarray(res.results[i]["out"]).astype(np.float32) for i in range(B)],
        axis=0,
    )
    return out.reshape(B, C, H, W)


if __name__ == "__main__":
    rng = np.random.default_rng(0)
    img = rng.standard_normal((8, 128, 256, 256), dtype=np.float32)
    gt = rng.integers(0, NCLS, size=(8, 1, 256, 256), dtype=np.int32)
    out = kernel(img=img, gt=gt)
    print("out", out.shape, out.dtype)


# revision 66
# speedup vs baseline: 1.0108x; 1.0096x over previous
"""Trainium2 Bass kernel: per-(image, channel) class-mean replacement (segment mean + gather).

Input:  img [8, 128, 256, 256] f32, gt [8, 1, 256, 256] int32 (labels in [0, 21))
Output: out[b, c, h, w] = mean over pixels p of img[b, c, p] where gt[b, p] == gt[b, h, w]

Sharding: data-parallel over batch — each of the 8 NeuronCores processes one image.

Per-core algorithm (C=128 channels on partitions, HW=65536 pixels on free axis):
  Setup:    gt -> chunk-major gtT [128pix, 512chunk] via PE transposes;
            class-major one-hot planes stash[p, c*512+gc] = (gtT[p,gc]==c),
            built in 64-chunk spans: span 0 on DVE (gates the first sums
            matmul); spans 1-7 split DVE (in-loop) / Pool (upfront).
  Phase 1:  PE-transpose img chunks; copy PSUM->SBUF with
            ->bf16 cast (DVE/Act alternating); sums matmul SWAPPED: stationary =
            imgT chunk [128px,128ch], moving = one-hot view [128px,21cls] ->
            accumulate sumsT[128ch,21cls] in PSUM (21-col outputs are nearly
            free). Counts via lhsT=onehot, rhs=ones -> cnt[21,1]. Sums matmuls
            issue two 512-px groups late (software pipelining) so the in-order
            PE queue never blocks on the copies. Phase-2 ohT pre-transposes for
            the first PRE_G groups are interleaved (Pool joins the copy
            rotation only after its span work is done).
  Means:    sumsT -> SBUF -> PE-transpose -> meansT[21,128] bf16 = sums*rcp(cnt).
  Phase 2:  out[128ch,512px] = meansT^T @ ohT[21,512] per group; copy PSUM->SBUF
            as bf16; DMA out 2048-px tiles. Output DRAM tensor is bf16 (host
            casts back to f32) — halves write bandwidth at zero added error
            since means are already bf16. Pre-transposed and JIT output tiles
            alternate 1:1; JIT ohT pairs are transposed two tiles ahead.
            NOTE: GPSIMD/Pool must never read PSUM (walrus rejects it), so all
            PSUM->SBUF copies stay on DVE/Act.
"""

import os
import sys

for _p in ("/opt/trn_rl_repo", "/root/.axon_site/_ro/trn_rl_repo"):
    if os.path.isdir(_p) and _p not in sys.path:
        sys.path.append(_p)

import numpy as np

P = 128          # channels == SBUF partitions
HW = 256 * 256   # pixels per image
NCLS = 21
CH = 128         # pixels per matmul chunk
NCH = HW // CH   # 512 chunks
FB = 2048        # pixels per DMA tile
NB = HW // FB    # 32 big tiles
NGR = HW // 512  # 128 phase-2 groups (512 px each)
PRE_G = 92       # groups whose ohT is pre-transposed during phase 1
EPS = 1e-8
N_CORES = 8

_CACHE = {}


def _build_module():
    import concourse.bacc as bacc
    import concourse.mybir as mybir
    import concourse.tile as tile
    from concourse.masks import make_identity

    f32 = mybir.dt.float32
    bf16 = mybir.dt.bfloat16
    i32 = mybir.dt.int32
    EQ = mybir.AluOpType.is_equal
    MULT = mybir.AluOpType.mult

    nc = bacc.Bacc("TRN2", target_bir_lowering=False, debug=False)
    img = nc.dram_tensor("img", [P, HW], f32, kind="ExternalInput")
    gt = nc.dram_tensor("gt", [HW], i32, kind="ExternalInput")
    out = nc.dram_tensor("out", [P, HW], bf16, kind="ExternalOutput")

    with tile.TileContext(nc) as tc:
        with (
            tc.tile_pool(name="constp", bufs=1) as constp,
            tc.tile_pool(name="imgp", bufs=5) as imgp,
            tc.tile_pool(name="rhsp", bufs=10) as rhsp,
            tc.tile_pool(name="ohsbp", bufs=2) as ohsbp,
            tc.tile_pool(name="outp", bufs=5) as outp,
            tc.tile_pool(name="psA", bufs=5, space="PSUM") as psA,
            tc.tile_pool(name="psB", bufs=1, space="PSUM") as psB,
            tc.tile_pool(name="psC", bufs=2, space="PSUM") as psC,
        ):
            # ---- constants ----
            ident32 = constp.tile([P, P], f32, tag="id32")
            make_identity(nc, ident32[:])
            ident16 = constp.tile([P, P], bf16, tag="id16")
            nc.vector.tensor_copy(out=ident16[:], in_=ident32[:])
            ones1 = constp.tile([P, 1], bf16, tag="ones1")
            nc.vector.memset(ones1[:], 1.0)

            # gt: load [32, 2048], cast f32 (Act), PE-transpose 16 blocks into
            # chunk-major gtT (block b holds chunks {16r+b}; stride-16 dest AP).
            # gt staging borrows imgp slots (same per-partition footprint).
            gtn_i = imgp.tile([32, HW // 32], i32, tag="img")
            gtn = imgp.tile([32, HW // 32], f32, tag="img")
            # gt loads FIRST on the SP queue (ahead of the img stream on the
            # serial DMA engines) in 2 pieces, casts pipelined on Act
            gt_pc = (HW // 32) // 2
            for pc in range(2):
                nc.sync.dma_start(
                    out=gtn_i[:, pc * gt_pc : (pc + 1) * gt_pc],
                    in_=gt.ap().rearrange("(p f) -> p f", p=32)[
                        :, pc * gt_pc : (pc + 1) * gt_pc
                    ],
                )
            for pc in range(2):
                nc.scalar.copy(
                    out=gtn[:, pc * gt_pc : (pc + 1) * gt_pc],
                    in_=gtn_i[:, pc * gt_pc : (pc + 1) * gt_pc],
                )
            gtT = constp.tile([P, NCH], f32, tag="gtT")
            gtTv = gtT[:].rearrange("p (r b) -> p r b", b=16)
            # all 16 [32,128]->[128,32] block transposes land in ONE psA tile,
            # then a single strided copy scatters them into chunk-major gtT —
            # avoids 16 cross-engine sem round-trips through a 2-deep pool
            gps16 = psA.tile([P, 512], f32, tag="a")
            for b in range(16):
                nc.tensor.transpose(
                    out=gps16[:, b * 32 : (b + 1) * 32],
                    in_=gtn[:, b * P : (b + 1) * P],
                    identity=ident32[0:32, 0:32],
                )
            nc.vector.tensor_copy(
                out=gtTv[:, :, :],
                in_=gps16[:].rearrange("p (b r) -> p r b", b=16),
            )

            # class-major one-hot planes: stash[p, c*NCH + gc] = (gtT[p,gc]==c)
            stash = constp.tile([P, NCLS * NCH], bf16, tag="stash")

            def issue_span(s, eng):
                for c in range(NCLS):
                    eng.tensor_scalar(
                        stash[:, c * NCH + 64 * s : c * NCH + 64 * (s + 1)],
                        gtT[:, 64 * s : 64 * (s + 1)],
                        float(c),
                        None,
                        EQ,
                    )

            # span 0 on DVE (it gates the first sums matmul). Spans 1-7 are
            # split by class: Pool halves issue upfront (Pool is idle), DVE
            # halves issue from inside the tile loop so the rhs copies are not
            # queued behind them — span s gates only tiles 4s and later.
            issue_span(0, nc.vector)

            def issue_span_half(s, eng_id):
                for c in range(NCLS):
                    if c % 2 == eng_id:
                        continue
                    eng = nc.vector if c % 2 == 0 else nc.gpsimd
                    eng.tensor_scalar(
                        stash[:, c * NCH + 64 * s : c * NCH + 64 * (s + 1)],
                        gtT[:, 64 * s : 64 * (s + 1)],
                        float(c),
                        None,
                        EQ,
                    )

            for s in range(1, 8):
                issue_span_half(s, 0)  # odd classes -> Pool, upfront
            stashv = stash[:].rearrange("p (c j) -> p c j", c=NCLS)

            def ohview(gc):
                return stashv[:, :, gc]  # [128px, 21cls]

            # pre-transposed ohT for groups [0, PRE_G)
            ohstash = constp.tile([32, PRE_G * 512], bf16, tag="ohstash")

            sums = psB.tile([P, NCLS], f32, tag="sums")
            # counts: 21 full-tile (XYZWC) reductions of the one-hot planes on
            # Pool into a [1,21] row (partition 0) — no PSUM bank, no PE work
            cntrow = constp.tile([1, NCLS], f32, tag="cntrow")

            def copy_by(eng, dst, src):
                if eng == 0:
                    nc.vector.tensor_copy(out=dst, in_=src)
                elif eng == 1:
                    nc.scalar.copy(out=dst, in_=src)
                else:
                    nc.gpsimd.tensor_copy(out=dst, in_=src)

            def pre_transpose_group(g, eng):
                ohps = psC.tile([32, 512], bf16, tag="c")
                for q in range(4):
                    nc.tensor.transpose(
                        out=ohps[0:NCLS, q * CH : (q + 1) * CH],
                        in_=ohview(g * 4 + q),
                        identity=ident16[:],
                    )
                copy_by(eng, ohstash[0:NCLS, g * 512 : (g + 1) * 512], ohps[0:NCLS, :])

            # ---- phase 1 ----
            LAG = 2
            pre_done = 0
            pending = []

            def issue_sums(g4, rhs4):
                for q in range(4):
                    gc = g4 * 4 + q
                    nc.tensor.matmul(
                        out=sums[:],
                        lhsT=rhs4[:, q * CH : (q + 1) * CH],
                        rhs=ohview(gc),
                        start=(gc == 0),
                        stop=(gc == NCH - 1),
                    )

            pre_done = 0
            for t in range(NB):
                ib = imgp.tile([P, FB], f32, tag="img")
                for h in range(2):
                    nc.sync.dma_start(
                        out=ib[:, h * 1024 : (h + 1) * 1024],
                        in_=img.ap()[:, t * FB + h * 1024 : t * FB + (h + 1) * 1024],
                    )
                for jj in range(4):
                    g4 = t * 4 + jj
                    tp4 = psA.tile([P, 512], f32, tag="a")
                    for q in range(4):
                        nc.tensor.transpose(
                            out=tp4[:, q * CH : (q + 1) * CH],
                            in_=ib[:, (jj * 4 + q) * CH : (jj * 4 + q + 1) * CH],
                            identity=ident32[:],
                        )
                    rhs4 = rhsp.tile([P, 512], bf16, tag="rhs")
                    copy_by(g4 % 2, rhs4[:], tp4[:])
                    pending.append((g4, rhs4))
                    if len(pending) > LAG:
                        issue_sums(*pending.pop(0))
                # DVE half of span t//4+1 right after this tile's copies
                if t % 4 == 0 and t // 4 + 1 < 8:
                    issue_span_half(t // 4 + 1, 1)  # even classes -> DVE
                # two phase-2 ohT pre-transposes per tile; Pool joins the
                # copy rotation only once its span building is long done
                target = min(PRE_G // 2, ((t + 1) * (PRE_G // 2)) // NB)
                while pre_done < target:
                    pre_transpose_group(2 * pre_done, pre_done % 2)
                    pre_transpose_group(2 * pre_done + 1, (pre_done + 1) % 2)
                    pre_done += 1
            while pending:
                issue_sums(*pending.pop(0))

            # counts: odd classes first (their planes finish on Pool early),
            # even classes after the in-loop DVE span halves complete
            for c in [c for c in range(NCLS) if c % 2] + [c for c in range(NCLS) if c % 2 == 0]:
                nc.gpsimd.tensor_reduce(
                    out=cntrow[0:1, c : c + 1],
                    in_=stash[:, c * NCH : (c + 1) * NCH],
                    axis=mybir.AxisListType.XYZWC,
                    op=mybir.AluOpType.add,
                )

            # ---- means: meansT[21,128] bf16 = sumsT^T * 1/(cnt+eps) ----
            cntP = psC.tile([NCLS, 1], f32, tag="c")
            nc.tensor.transpose(
                out=cntP[:], in_=cntrow[:], identity=ident32[0:1, 0:1]
            )
            cnte = constp.tile([NCLS, 1], f32, tag="cnte")
            nc.vector.tensor_scalar_add(cnte[:], cntP[:], EPS)
            rcp = constp.tile([NCLS, 1], f32, tag="rcp")
            nc.vector.reciprocal(out=rcp[:], in_=cnte[:])
            sms = constp.tile([P, NCLS], f32, tag="sms")
            nc.vector.tensor_copy(out=sms[:], in_=sums[:])
            smsP = psC.tile([NCLS, P], f32, tag="c")
            nc.tensor.transpose(out=smsP[:], in_=sms[:], identity=ident32[:])
            meansT = constp.tile([NCLS, P], bf16, tag="meansT")
            nc.vector.tensor_scalar(meansT[:], smsP[:], rcp[:, 0:1], None, MULT)

            # ---- phase 2: out[128ch, px] = meansT^T @ ohT ----
            # Pre-transposed and JIT output tiles alternate 1:1; each JIT
            # tile's two ohT pairs are transposed+copied one position ahead.
            n_pre_t = PRE_G // 4
            n_jit = NB - n_pre_t
            tile_order, pi, ji, err = [], 0, n_pre_t, 0
            for k in range(NB):
                err += n_jit
                if err >= NB and ji < NB and pi > 0:
                    tile_order.append(ji)
                    ji += 1
                    err -= NB
                else:
                    tile_order.append(pi)
                    pi += 1
            jit_ohs = {}
            jit_cnt = [0]

            def stage_jit(tt):
                pair = []
                for half in range(2):
                    ohps2 = psC.tile([32, 1024], bf16, tag="c")
                    for qq in range(8):
                        nc.tensor.transpose(
                            out=ohps2[0:NCLS, qq * CH : (qq + 1) * CH],
                            in_=ohview((4 * tt + 2 * half) * 4 + qq),
                            identity=ident16[:],
                        )
                    ohs = ohsbp.tile([32, 1024], bf16, tag="oh")
                    copy_by(jit_cnt[0] % 2, ohs[0:NCLS, :], ohps2[0:NCLS, :])
                    jit_cnt[0] += 1
                    pair.append(ohs)
                jit_ohs[tt] = pair

            # stage each JIT tile's ohT pairs two positions ahead
            if len(tile_order) > 1 and tile_order[1] >= n_pre_t:
                stage_jit(tile_order[1])
            for pos, tt in enumerate(tile_order):
                if pos + 2 < len(tile_order) and tile_order[pos + 2] >= n_pre_t:
                    stage_jit(tile_order[pos + 2])
                jit = tt >= n_pre_t
                ob4 = outp.tile([P, FB], bf16, tag="ob")
                for k in range(4):
                    g = 4 * tt + k
                    if jit:
                        rhs_ap = jit_ohs[tt][k // 2][0:NCLS, (k % 2) * 512 : (k % 2 + 1) * 512]
                    else:
                        rhs_ap = ohstash[0:NCLS, g * 512 : (g + 1) * 512]
                    op_ = psA.tile([P, 512], f32, tag="a")
                    nc.tensor.matmul(
                        out=op_[:], lhsT=meansT[:], rhs=rhs_ap, start=True, stop=True
                    )
                    rot = ((1, 0, 1, 0), (0, 1, 0, 1))[pos % 2]
                    copy_by(rot[k], ob4[:, k * 512 : (k + 1) * 512], op_[:])
                if pos == 0 or pos == len(tile_order) - 1:
                    # split first tiles' DMAs (stream starts sooner after
                    # means) and the last tile's (tail drains sooner)
                    for s in range(4):
                        nc.sync.dma_start(
                            out=out.ap()[:, (4 * tt + s) * 512 : (4 * tt + s + 1) * 512],
                            in_=ob4[:, s * 512 : (s + 1) * 512],
                        )
                else:
                    nc.sync.dma_start(
                        out=out.ap()[:, tt * FB : (tt + 1) * FB], in_=ob4[:]
                    )

    nc.compile()
    return nc


def get_module():
    if "nc" not in _CACHE:
        _CACHE["nc"] = _build_module()
    return _CACHE["nc"]


def kernel(img, gt):
    from concourse.bass_utils import run_bass_kernel_spmd

    img = np.asarray(img)
    gt = np.asarray(gt)
    B, C, H, W = img.shape
    assert (B, C, H * W) == (N_CORES, P, HW), (img.shape,)
    img2 = np.ascontiguousarray(img.reshape(B, C, H * W))
    gt2 = np.ascontiguousarray(gt.reshape(B, H * W))

    nc = get_module()
    in_maps = [{"img": img2[i], "gt": gt2[i]} for i in range(B)]
    res = run_bass_kernel_spmd(nc, in_maps, core_ids=list(range(N_CORES)))
    out = np.stack(
        [np.asarray(res.results[i]["out"]).astype(np.float32) for i in range(B)],
        axis=0,
    )
    return out.reshape(B, C, H, W)


if __name__ == "__main__":
    rng = np.random.default_rng(0)
    img = rng.standard_normal((8, 128, 256, 256), dtype=np.float32)
    gt = rng.integers(0, NCLS, size=(8, 1, 256, 256), dtype=np.int32)
    out = kernel(img=img, gt=gt)
    print("out", out.shape, out.dtype)


# revision 67
# speedup vs baseline: 1.0114x; 1.0005x over previous
"""Trainium2 Bass kernel: per-(image, channel) class-mean replacement (segment mean + gather).

Input:  img [8, 128, 256, 256] f32, gt [8, 1, 256, 256] int32 (labels in [0, 21))
Output: out[b, c, h, w] = mean over pixels p of img[b, c, p] where gt[b, p] == gt[b, h, w]

Sharding: data-parallel over batch — each of the 8 NeuronCores processes one image.

Per-core algorithm (C=128 channels on partitions, HW=65536 pixels on free axis):
  Setup:    gt -> chunk-major gtT [128pix, 512chunk] via PE transposes;
            class-major one-hot planes stash[p, c*512+gc] = (gtT[p,gc]==c),
            built in 64-chunk spans: span 0 on DVE (gates the first sums
            matmul); spans 1-7 split DVE (in-loop) / Pool (upfront).
  Phase 1:  PE-transpose img chunks; copy PSUM->SBUF with
            ->bf16 cast (DVE/Act alternating); sums matmul SWAPPED: stationary =
            imgT chunk [128px,128ch], moving = one-hot view [128px,21cls] ->
            accumulate sumsT[128ch,21cls] in PSUM (21-col outputs are nearly
            free). Counts via lhsT=onehot, rhs=ones -> cnt[21,1]. Sums matmuls
            issue two 512-px groups late (software pipelining) so the in-order
            PE queue never blocks on the copies. Phase-2 ohT pre-transposes for
            the first PRE_G groups are interleaved (Pool joins the copy
            rotation only after its span work is done).
  Means:    sumsT -> SBUF -> PE-transpose -> meansT[21,128] bf16 = sums*rcp(cnt).
  Phase 2:  out[128ch,512px] = meansT^T @ ohT[21,512] per group; copy PSUM->SBUF
            as bf16; DMA out 2048-px tiles. Output DRAM tensor is bf16 (host
            casts back to f32) — halves write bandwidth at zero added error
            since means are already bf16. Pre-transposed and JIT output tiles
            alternate 1:1; JIT ohT pairs are transposed two tiles ahead.
            NOTE: GPSIMD/Pool must never read PSUM (walrus rejects it), so all
            PSUM->SBUF copies stay on DVE/Act.
"""

import os
import sys

for _p in ("/opt/trn_rl_repo", "/root/.axon_site/_ro/trn_rl_repo"):
    if os.path.isdir(_p) and _p not in sys.path:
        sys.path.append(_p)

import numpy as np

P = 128          # channels == SBUF partitions
HW = 256 * 256   # pixels per image
NCLS = 21
CH = 128         # pixels per matmul chunk
NCH = HW // CH   # 512 chunks
FB = 2048        # pixels per DMA tile
NB = HW // FB    # 32 big tiles
NGR = HW // 512  # 128 phase-2 groups (512 px each)
PRE_G = 92       # groups whose ohT is pre-transposed during phase 1
EPS = 1e-8
N_CORES = 8

_CACHE = {}


def _build_module():
    import concourse.bacc as bacc
    import concourse.mybir as mybir
    import concourse.tile as tile
    from concourse.masks import make_identity

    f32 = mybir.dt.float32
    bf16 = mybir.dt.bfloat16
    i32 = mybir.dt.int32
    EQ = mybir.AluOpType.is_equal
    MULT = mybir.AluOpType.mult

    nc = bacc.Bacc("TRN2", target_bir_lowering=False, debug=False)
    img = nc.dram_tensor("img", [P, HW], f32, kind="ExternalInput")
    gt = nc.dram_tensor("gt", [HW], i32, kind="ExternalInput")
    out = nc.dram_tensor("out", [P, HW], bf16, kind="ExternalOutput")

    with tile.TileContext(nc) as tc:
        with (
            tc.tile_pool(name="constp", bufs=1) as constp,
            tc.tile_pool(name="imgp", bufs=5) as imgp,
            tc.tile_pool(name="rhsp", bufs=12) as rhsp,
            tc.tile_pool(name="ohsbp", bufs=2) as ohsbp,
            tc.tile_pool(name="outp", bufs=5) as outp,
            tc.tile_pool(name="psA", bufs=5, space="PSUM") as psA,
            tc.tile_pool(name="psB", bufs=1, space="PSUM") as psB,
            tc.tile_pool(name="psC", bufs=2, space="PSUM") as psC,
        ):
            # ---- constants ----
            ident32 = constp.tile([P, P], f32, tag="id32")
            make_identity(nc, ident32[:])
            ident16 = constp.tile([P, P], bf16, tag="id16")
            nc.vector.tensor_copy(out=ident16[:], in_=ident32[:])
            ones1 = constp.tile([P, 1], bf16, tag="ones1")
            nc.vector.memset(ones1[:], 1.0)

            # gt: load [32, 2048], cast f32 (Act), PE-transpose 16 blocks into
            # chunk-major gtT (block b holds chunks {16r+b}; stride-16 dest AP).
            # gt staging borrows imgp slots (same per-partition footprint).
            gtn_i = imgp.tile([32, HW // 32], i32, tag="img")
            gtn = imgp.tile([32, HW // 32], f32, tag="img")
            # gt loads FIRST on the SP queue (ahead of the img stream on the
            # serial DMA engines) in 2 pieces, casts pipelined on Act
            gt_pc = (HW // 32) // 2
            for pc in range(2):
                nc.sync.dma_start(
                    out=gtn_i[:, pc * gt_pc : (pc + 1) * gt_pc],
                    in_=gt.ap().rearrange("(p f) -> p f", p=32)[
                        :, pc * gt_pc : (pc + 1) * gt_pc
                    ],
                )
            for pc in range(2):
                nc.scalar.copy(
                    out=gtn[:, pc * gt_pc : (pc + 1) * gt_pc],
                    in_=gtn_i[:, pc * gt_pc : (pc + 1) * gt_pc],
                )
            gtT = constp.tile([P, NCH], f32, tag="gtT")
            gtTv = gtT[:].rearrange("p (r b) -> p r b", b=16)
            # all 16 [32,128]->[128,32] block transposes land in ONE psA tile,
            # then a single strided copy scatters them into chunk-major gtT —
            # avoids 16 cross-engine sem round-trips through a 2-deep pool
            gps16 = psA.tile([P, 512], f32, tag="a")
            for b in range(16):
                nc.tensor.transpose(
                    out=gps16[:, b * 32 : (b + 1) * 32],
                    in_=gtn[:, b * P : (b + 1) * P],
                    identity=ident32[0:32, 0:32],
                )
            nc.vector.tensor_copy(
                out=gtTv[:, :, :],
                in_=gps16[:].rearrange("p (b r) -> p r b", b=16),
            )

            # class-major one-hot planes: stash[p, c*NCH + gc] = (gtT[p,gc]==c)
            stash = constp.tile([P, NCLS * NCH], bf16, tag="stash")

            def issue_span(s, eng):
                for c in range(NCLS):
                    eng.tensor_scalar(
                        stash[:, c * NCH + 64 * s : c * NCH + 64 * (s + 1)],
                        gtT[:, 64 * s : 64 * (s + 1)],
                        float(c),
                        None,
                        EQ,
                    )

            # span 0 on DVE (it gates the first sums matmul). Spans 1-7 are
            # split by class: Pool halves issue upfront (Pool is idle), DVE
            # halves issue from inside the tile loop so the rhs copies are not
            # queued behind them — span s gates only tiles 4s and later.
            issue_span(0, nc.vector)

            def issue_span_half(s, eng_id):
                for c in range(NCLS):
                    if c % 2 == eng_id:
                        continue
                    eng = nc.vector if c % 2 == 0 else nc.gpsimd
                    eng.tensor_scalar(
                        stash[:, c * NCH + 64 * s : c * NCH + 64 * (s + 1)],
                        gtT[:, 64 * s : 64 * (s + 1)],
                        float(c),
                        None,
                        EQ,
                    )

            for s in range(1, 8):
                issue_span_half(s, 0)  # odd classes -> Pool, upfront
            stashv = stash[:].rearrange("p (c j) -> p c j", c=NCLS)

            def ohview(gc):
                return stashv[:, :, gc]  # [128px, 21cls]

            # pre-transposed ohT for groups [0, PRE_G)
            ohstash = constp.tile([32, PRE_G * 512], bf16, tag="ohstash")

            sums = psB.tile([P, NCLS], f32, tag="sums")
            # counts: 21 full-tile (XYZWC) reductions of the one-hot planes on
            # Pool into a [1,21] row (partition 0) — no PSUM bank, no PE work
            cntrow = constp.tile([1, NCLS], f32, tag="cntrow")

            def copy_by(eng, dst, src):
                if eng == 0:
                    nc.vector.tensor_copy(out=dst, in_=src)
                elif eng == 1:
                    nc.scalar.copy(out=dst, in_=src)
                else:
                    nc.gpsimd.tensor_copy(out=dst, in_=src)

            def pre_transpose_group(g, eng):
                ohps = psC.tile([32, 512], bf16, tag="c")
                for q in range(4):
                    nc.tensor.transpose(
                        out=ohps[0:NCLS, q * CH : (q + 1) * CH],
                        in_=ohview(g * 4 + q),
                        identity=ident16[:],
                    )
                copy_by(eng, ohstash[0:NCLS, g * 512 : (g + 1) * 512], ohps[0:NCLS, :])

            # ---- phase 1 ----
            LAG = 2
            pre_done = 0
            pending = []

            def issue_sums(g4, rhs4):
                for q in range(4):
                    gc = g4 * 4 + q
                    nc.tensor.matmul(
                        out=sums[:],
                        lhsT=rhs4[:, q * CH : (q + 1) * CH],
                        rhs=ohview(gc),
                        start=(gc == 0),
                        stop=(gc == NCH - 1),
                    )

            pre_done = 0
            for t in range(NB):
                ib = imgp.tile([P, FB], f32, tag="img")
                for h in range(2):
                    nc.sync.dma_start(
                        out=ib[:, h * 1024 : (h + 1) * 1024],
                        in_=img.ap()[:, t * FB + h * 1024 : t * FB + (h + 1) * 1024],
                    )
                for jj in range(4):
                    g4 = t * 4 + jj
                    tp4 = psA.tile([P, 512], f32, tag="a")
                    for q in range(4):
                        nc.tensor.transpose(
                            out=tp4[:, q * CH : (q + 1) * CH],
                            in_=ib[:, (jj * 4 + q) * CH : (jj * 4 + q + 1) * CH],
                            identity=ident32[:],
                        )
                    rhs4 = rhsp.tile([P, 512], bf16, tag="rhs")
                    copy_by(g4 % 2, rhs4[:], tp4[:])
                    pending.append((g4, rhs4))
                    if len(pending) > LAG:
                        issue_sums(*pending.pop(0))
                # DVE half of span t//4+1 right after this tile's copies
                if t % 4 == 0 and t // 4 + 1 < 8:
                    issue_span_half(t // 4 + 1, 1)  # even classes -> DVE
                # two phase-2 ohT pre-transposes per tile; Pool joins the
                # copy rotation only once its span building is long done
                target = min(PRE_G // 2, ((t + 1) * (PRE_G // 2)) // NB)
                while pre_done < target:
                    pre_transpose_group(2 * pre_done, pre_done % 2)
                    pre_transpose_group(2 * pre_done + 1, (pre_done + 1) % 2)
                    pre_done += 1
            while pending:
                issue_sums(*pending.pop(0))

            # counts: odd classes first (their planes finish on Pool early),
            # even classes after the in-loop DVE span halves complete
            for c in [c for c in range(NCLS) if c % 2] + [c for c in range(NCLS) if c % 2 == 0]:
                nc.gpsimd.tensor_reduce(
                    out=cntrow[0:1, c : c + 1],
                    in_=stash[:, c * NCH : (c + 1) * NCH],
                    axis=mybir.AxisListType.XYZWC,
                    op=mybir.AluOpType.add,
                )

            # ---- means: meansT[21,128] bf16 = sumsT^T * 1/(cnt+eps) ----
            cntP = psC.tile([NCLS, 1], f32, tag="c")
            nc.tensor.transpose(
                out=cntP[:], in_=cntrow[:], identity=ident32[0:1, 0:1]
            )
            cnte = constp.tile([NCLS, 1], f32, tag="cnte")
            nc.vector.tensor_scalar_add(cnte[:], cntP[:], EPS)
            rcp = constp.tile([NCLS, 1], f32, tag="rcp")
            nc.vector.reciprocal(out=rcp[:], in_=cnte[:])
            sms = constp.tile([P, NCLS], f32, tag="sms")
            nc.vector.tensor_copy(out=sms[:], in_=sums[:])
            smsP = psC.tile([NCLS, P], f32, tag="c")
            nc.tensor.transpose(out=smsP[:], in_=sms[:], identity=ident32[:])
            meansT = constp.tile([NCLS, P], bf16, tag="meansT")
            nc.vector.tensor_scalar(meansT[:], smsP[:], rcp[:, 0:1], None, MULT)

            # ---- phase 2: out[128ch, px] = meansT^T @ ohT ----
            # Pre-transposed and JIT output tiles alternate 1:1; each JIT
            # tile's two ohT pairs are transposed+copied one position ahead.
            n_pre_t = PRE_G // 4
            n_jit = NB - n_pre_t
            tile_order, pi, ji, err = [], 0, n_pre_t, 0
            for k in range(NB):
                err += n_jit
                if err >= NB and ji < NB and pi > 0:
                    tile_order.append(ji)
                    ji += 1
                    err -= NB
                else:
                    tile_order.append(pi)
                    pi += 1
            jit_ohs = {}
            jit_cnt = [0]

            def stage_jit(tt):
                pair = []
                for half in range(2):
                    ohps2 = psC.tile([32, 1024], bf16, tag="c")
                    for qq in range(8):
                        nc.tensor.transpose(
                            out=ohps2[0:NCLS, qq * CH : (qq + 1) * CH],
                            in_=ohview((4 * tt + 2 * half) * 4 + qq),
                            identity=ident16[:],
                        )
                    ohs = ohsbp.tile([32, 1024], bf16, tag="oh")
                    copy_by(jit_cnt[0] % 2, ohs[0:NCLS, :], ohps2[0:NCLS, :])
                    jit_cnt[0] += 1
                    pair.append(ohs)
                jit_ohs[tt] = pair

            # stage each JIT tile's ohT pairs two positions ahead
            if len(tile_order) > 1 and tile_order[1] >= n_pre_t:
                stage_jit(tile_order[1])
            for pos, tt in enumerate(tile_order):
                if pos + 2 < len(tile_order) and tile_order[pos + 2] >= n_pre_t:
                    stage_jit(tile_order[pos + 2])
                jit = tt >= n_pre_t
                ob4 = outp.tile([P, FB], bf16, tag="ob")
                for k in range(4):
                    g = 4 * tt + k
                    if jit:
                        rhs_ap = jit_ohs[tt][k // 2][0:NCLS, (k % 2) * 512 : (k % 2 + 1) * 512]
                    else:
                        rhs_ap = ohstash[0:NCLS, g * 512 : (g + 1) * 512]
                    op_ = psA.tile([P, 512], f32, tag="a")
                    nc.tensor.matmul(
                        out=op_[:], lhsT=meansT[:], rhs=rhs_ap, start=True, stop=True
                    )
                    rot = ((1, 0, 1, 0), (0, 1, 0, 1))[pos % 2]
                    copy_by(rot[k], ob4[:, k * 512 : (k + 1) * 512], op_[:])
                if pos == 0 or pos == len(tile_order) - 1:
                    # split first tiles' DMAs (stream starts sooner after
                    # means) and the last tile's (tail drains sooner)
                    for s in range(4):
                        nc.sync.dma_start(
                            out=out.ap()[:, (4 * tt + s) * 512 : (4 * tt + s + 1) * 512],
                            in_=ob4[:, s * 512 : (s + 1) * 512],
                        )
                else:
                    nc.sync.dma_start(
                        out=out.ap()[:, tt * FB : (tt + 1) * FB], in_=ob4[:]
                    )

    nc.compile()
    return nc


def get_module():
    if "nc" not in _CACHE:
        _CACHE["nc"] = _build_module()
    return _CACHE["nc"]


def kernel(img, gt):
    from concourse.bass_utils import run_bass_kernel_spmd

    img = np.asarray(img)
    gt = np.asarray(gt)
    B, C, H, W = img.shape
    assert (B, C, H * W) == (N_CORES, P, HW), (img.shape,)
    img2 = np.ascontiguousarray(img.reshape(B, C, H * W))
    gt2 = np.ascontiguousarray(gt.reshape(B, H * W))

    nc = get_module()
    in_maps = [{"img": img2[i], "gt": gt2[i]} for i in range(B)]
    res = run_bass_kernel_spmd(nc, in_maps, core_ids=list(range(N_CORES)))
    out = np.stack(
        [np.asarray(res.results[i]["out"]).astype(np.float32) for i in range(B)],
        axis=0,
    )
    return out.reshape(B, C, H, W)


if __name__ == "__main__":
    rng = np.random.default_rng(0)
    img = rng.standard_normal((8, 128, 256, 256), dtype=np.float32)
    gt = rng.integers(0, NCLS, size=(8, 1, 256, 256), dtype=np.int32)
    out = kernel(img=img, gt=gt)
    print("out", out.shape, out.dtype)


# revision 68
# speedup vs baseline: 1.0127x; 1.0013x over previous
"""Trainium2 Bass kernel: per-(image, channel) class-mean replacement (segment mean + gather).

Input:  img [8, 128, 256, 256] f32, gt [8, 1, 256, 256] int32 (labels in [0, 21))
Output: out[b, c, h, w] = mean over pixels p of img[b, c, p] where gt[b, p] == gt[b, h, w]

Sharding: data-parallel over batch — each of the 8 NeuronCores processes one image.

Per-core algorithm (C=128 channels on partitions, HW=65536 pixels on free axis):
  Setup:    gt -> chunk-major gtT [128pix, 512chunk] via PE transposes;
            class-major one-hot planes stash[p, c*512+gc] = (gtT[p,gc]==c),
            built in 64-chunk spans: span 0 on DVE (gates the first sums
            matmul); spans 1-7 split DVE (in-loop) / Pool (upfront).
  Phase 1:  PE-transpose img chunks; copy PSUM->SBUF with
            ->bf16 cast (DVE/Act alternating); sums matmul SWAPPED: stationary =
            imgT chunk [128px,128ch], moving = one-hot view [128px,21cls] ->
            accumulate sumsT[128ch,21cls] in PSUM (21-col outputs are nearly
            free). Counts via lhsT=onehot, rhs=ones -> cnt[21,1]. Sums matmuls
            issue two 512-px groups late (software pipelining) so the in-order
            PE queue never blocks on the copies. Phase-2 ohT pre-transposes for
            the first PRE_G groups are interleaved (Pool joins the copy
            rotation only after its span work is done).
  Means:    sumsT -> SBUF -> PE-transpose -> meansT[21,128] bf16 = sums*rcp(cnt).
  Phase 2:  out[128ch,512px] = meansT^T @ ohT[21,512] per group; copy PSUM->SBUF
            as bf16; DMA out 2048-px tiles. Output DRAM tensor is bf16 (host
            casts back to f32) — halves write bandwidth at zero added error
            since means are already bf16. Pre-transposed and JIT output tiles
            alternate 1:1; JIT ohT pairs are transposed two tiles ahead.
            NOTE: GPSIMD/Pool must never read PSUM (walrus rejects it), so all
            PSUM->SBUF copies stay on DVE/Act.
"""

import os
import sys

for _p in ("/opt/trn_rl_repo", "/root/.axon_site/_ro/trn_rl_repo"):
    if os.path.isdir(_p) and _p not in sys.path:
        sys.path.append(_p)

import numpy as np

P = 128          # channels == SBUF partitions
HW = 256 * 256   # pixels per image
NCLS = 21
CH = 128         # pixels per matmul chunk
NCH = HW // CH   # 512 chunks
FB = 2048        # pixels per DMA tile
NB = HW // FB    # 32 big tiles
NGR = HW // 512  # 128 phase-2 groups (512 px each)
PRE_G = 92       # groups whose ohT is pre-transposed during phase 1
EPS = 1e-8
N_CORES = 8

_CACHE = {}


def _build_module():
    import concourse.bacc as bacc
    import concourse.mybir as mybir
    import concourse.tile as tile
    from concourse.masks import make_identity

    f32 = mybir.dt.float32
    bf16 = mybir.dt.bfloat16
    i32 = mybir.dt.int32
    EQ = mybir.AluOpType.is_equal
    MULT = mybir.AluOpType.mult

    nc = bacc.Bacc("TRN2", target_bir_lowering=False, debug=False)
    img = nc.dram_tensor("img", [P, HW], f32, kind="ExternalInput")
    gt = nc.dram_tensor("gt", [HW], i32, kind="ExternalInput")
    out = nc.dram_tensor("out", [P, HW], bf16, kind="ExternalOutput")

    with tile.TileContext(nc) as tc:
        with (
            tc.tile_pool(name="constp", bufs=1) as constp,
            tc.tile_pool(name="imgp", bufs=5) as imgp,
            tc.tile_pool(name="rhsp", bufs=12) as rhsp,
            tc.tile_pool(name="ohsbp", bufs=2) as ohsbp,
            tc.tile_pool(name="outp", bufs=5) as outp,
            tc.tile_pool(name="psA", bufs=5, space="PSUM") as psA,
            tc.tile_pool(name="psB", bufs=1, space="PSUM") as psB,
            tc.tile_pool(name="psC", bufs=2, space="PSUM") as psC,
        ):
            # ---- constants ----
            ident32 = constp.tile([P, P], f32, tag="id32")
            make_identity(nc, ident32[:])
            ident16 = constp.tile([P, P], bf16, tag="id16")
            nc.vector.tensor_copy(out=ident16[:], in_=ident32[:])
            ones1 = constp.tile([P, 1], bf16, tag="ones1")
            nc.vector.memset(ones1[:], 1.0)

            # gt: load [32, 2048], cast f32 (Act), PE-transpose 16 blocks into
            # chunk-major gtT (block b holds chunks {16r+b}; stride-16 dest AP).
            # gt staging borrows imgp slots (same per-partition footprint).
            gtn_i = imgp.tile([32, HW // 32], i32, tag="img")
            gtn = imgp.tile([32, HW // 32], f32, tag="img")
            # gt loads FIRST on the SP queue (ahead of the img stream on the
            # serial DMA engines) in 2 pieces, casts pipelined on Act
            gt_pc = (HW // 32) // 2
            for pc in range(2):
                nc.sync.dma_start(
                    out=gtn_i[:, pc * gt_pc : (pc + 1) * gt_pc],
                    in_=gt.ap().rearrange("(p f) -> p f", p=32)[
                        :, pc * gt_pc : (pc + 1) * gt_pc
                    ],
                )
            for pc in range(2):
                nc.scalar.copy(
                    out=gtn[:, pc * gt_pc : (pc + 1) * gt_pc],
                    in_=gtn_i[:, pc * gt_pc : (pc + 1) * gt_pc],
                )
            gtT = constp.tile([P, NCH], f32, tag="gtT")
            gtTv = gtT[:].rearrange("p (r b) -> p r b", b=16)
            # all 16 [32,128]->[128,32] block transposes land in ONE psA tile,
            # then a single strided copy scatters them into chunk-major gtT —
            # avoids 16 cross-engine sem round-trips through a 2-deep pool
            gps16 = psA.tile([P, 512], f32, tag="a")
            for b in range(16):
                nc.tensor.transpose(
                    out=gps16[:, b * 32 : (b + 1) * 32],
                    in_=gtn[:, b * P : (b + 1) * P],
                    identity=ident32[0:32, 0:32],
                )
            nc.vector.tensor_copy(
                out=gtTv[:, :, :],
                in_=gps16[:].rearrange("p (b r) -> p r b", b=16),
            )

            # class-major one-hot planes: stash[p, c*NCH + gc] = (gtT[p,gc]==c)
            stash = constp.tile([P, NCLS * NCH], bf16, tag="stash")

            def issue_span(s, eng):
                for c in range(NCLS):
                    eng.tensor_scalar(
                        stash[:, c * NCH + 64 * s : c * NCH + 64 * (s + 1)],
                        gtT[:, 64 * s : 64 * (s + 1)],
                        float(c),
                        None,
                        EQ,
                    )

            # span 0 on DVE (it gates the first sums matmul). Spans 1-7 are
            # split by class: Pool halves issue upfront (Pool is idle), DVE
            # halves issue from inside the tile loop so the rhs copies are not
            # queued behind them — span s gates only tiles 4s and later.
            issue_span(0, nc.vector)

            def issue_span_half(s, eng_id):
                for c in range(NCLS):
                    if c % 2 == eng_id:
                        continue
                    eng = nc.vector if c % 2 == 0 else nc.gpsimd
                    eng.tensor_scalar(
                        stash[:, c * NCH + 64 * s : c * NCH + 64 * (s + 1)],
                        gtT[:, 64 * s : 64 * (s + 1)],
                        float(c),
                        None,
                        EQ,
                    )

            for s in range(1, 8):
                issue_span_half(s, 0)  # odd classes -> Pool, upfront
            stashv = stash[:].rearrange("p (c j) -> p c j", c=NCLS)

            def ohview(gc):
                return stashv[:, :, gc]  # [128px, 21cls]

            # pre-transposed ohT for groups [0, PRE_G)
            ohstash = constp.tile([32, PRE_G * 512], bf16, tag="ohstash")

            sums = psB.tile([P, NCLS], f32, tag="sums")
            # counts: 21 full-tile (XYZWC) reductions of the one-hot planes on
            # Pool into a [1,21] row (partition 0) — no PSUM bank, no PE work
            cntrow = constp.tile([1, NCLS], f32, tag="cntrow")

            def copy_by(eng, dst, src):
                if eng == 0:
                    nc.vector.tensor_copy(out=dst, in_=src)
                elif eng == 1:
                    nc.scalar.copy(out=dst, in_=src)
                else:
                    nc.gpsimd.tensor_copy(out=dst, in_=src)

            def pre_transpose_group(g, eng):
                ohps = psC.tile([32, 512], bf16, tag="c")
                for q in range(4):
                    nc.tensor.transpose(
                        out=ohps[0:NCLS, q * CH : (q + 1) * CH],
                        in_=ohview(g * 4 + q),
                        identity=ident16[:],
                    )
                copy_by(eng, ohstash[0:NCLS, g * 512 : (g + 1) * 512], ohps[0:NCLS, :])

            # ---- phase 1 ----
            LAG = 3
            pre_done = 0
            pending = []

            def issue_sums(g4, rhs4):
                for q in range(4):
                    gc = g4 * 4 + q
                    nc.tensor.matmul(
                        out=sums[:],
                        lhsT=rhs4[:, q * CH : (q + 1) * CH],
                        rhs=ohview(gc),
                        start=(gc == 0),
                        stop=(gc == NCH - 1),
                    )

            pre_done = 0
            for t in range(NB):
                ib = imgp.tile([P, FB], f32, tag="img")
                for h in range(2):
                    nc.sync.dma_start(
                        out=ib[:, h * 1024 : (h + 1) * 1024],
                        in_=img.ap()[:, t * FB + h * 1024 : t * FB + (h + 1) * 1024],
                    )
                for jj in range(4):
                    g4 = t * 4 + jj
                    tp4 = psA.tile([P, 512], f32, tag="a")
                    for q in range(4):
                        nc.tensor.transpose(
                            out=tp4[:, q * CH : (q + 1) * CH],
                            in_=ib[:, (jj * 4 + q) * CH : (jj * 4 + q + 1) * CH],
                            identity=ident32[:],
                        )
                    rhs4 = rhsp.tile([P, 512], bf16, tag="rhs")
                    copy_by(g4 % 2, rhs4[:], tp4[:])
                    pending.append((g4, rhs4))
                    if len(pending) > LAG:
                        issue_sums(*pending.pop(0))
                # DVE half of span t//4+1 right after this tile's copies
                if t % 4 == 0 and t // 4 + 1 < 8:
                    issue_span_half(t // 4 + 1, 1)  # even classes -> DVE
                # two phase-2 ohT pre-transposes per tile; Pool joins the
                # copy rotation only once its span building is long done
                target = min(PRE_G // 2, ((t + 1) * (PRE_G // 2)) // NB)
                while pre_done < target:
                    pre_transpose_group(2 * pre_done, pre_done % 2)
                    pre_transpose_group(2 * pre_done + 1, (pre_done + 1) % 2)
                    pre_done += 1
            while pending:
                issue_sums(*pending.pop(0))

            # counts: odd classes first (their planes finish on Pool early),
            # even classes after the in-loop DVE span halves complete
            for c in [c for c in range(NCLS) if c % 2] + [c for c in range(NCLS) if c % 2 == 0]:
                nc.gpsimd.tensor_reduce(
                    out=cntrow[0:1, c : c + 1],
                    in_=stash[:, c * NCH : (c + 1) * NCH],
                    axis=mybir.AxisListType.XYZWC,
                    op=mybir.AluOpType.add,
                )

            # ---- means: meansT[21,128] bf16 = sumsT^T * 1/(cnt+eps) ----
            cntP = psC.tile([NCLS, 1], f32, tag="c")
            nc.tensor.transpose(
                out=cntP[:], in_=cntrow[:], identity=ident32[0:1, 0:1]
            )
            cnte = constp.tile([NCLS, 1], f32, tag="cnte")
            nc.vector.tensor_scalar_add(cnte[:], cntP[:], EPS)
            rcp = constp.tile([NCLS, 1], f32, tag="rcp")
            nc.vector.reciprocal(out=rcp[:], in_=cnte[:])
            sms = constp.tile([P, NCLS], f32, tag="sms")
            nc.vector.tensor_copy(out=sms[:], in_=sums[:])
            smsP = psC.tile([NCLS, P], f32, tag="c")
            nc.tensor.transpose(out=smsP[:], in_=sms[:], identity=ident32[:])
            meansT = constp.tile([NCLS, P], bf16, tag="meansT")
            nc.vector.tensor_scalar(meansT[:], smsP[:], rcp[:, 0:1], None, MULT)

            # ---- phase 2: out[128ch, px] = meansT^T @ ohT ----
            # Pre-transposed and JIT output tiles alternate 1:1; each JIT
            # tile's two ohT pairs are transposed+copied one position ahead.
            n_pre_t = PRE_G // 4
            n_jit = NB - n_pre_t
            tile_order, pi, ji, err = [], 0, n_pre_t, 0
            for k in range(NB):
                err += n_jit
                if err >= NB and ji < NB and pi > 0:
                    tile_order.append(ji)
                    ji += 1
                    err -= NB
                else:
                    tile_order.append(pi)
                    pi += 1
            jit_ohs = {}
            jit_cnt = [0]

            def stage_jit(tt):
                pair = []
                for half in range(2):
                    ohps2 = psC.tile([32, 1024], bf16, tag="c")
                    for qq in range(8):
                        nc.tensor.transpose(
                            out=ohps2[0:NCLS, qq * CH : (qq + 1) * CH],
                            in_=ohview((4 * tt + 2 * half) * 4 + qq),
                            identity=ident16[:],
                        )
                    ohs = ohsbp.tile([32, 1024], bf16, tag="oh")
                    copy_by(jit_cnt[0] % 2, ohs[0:NCLS, :], ohps2[0:NCLS, :])
                    jit_cnt[0] += 1
                    pair.append(ohs)
                jit_ohs[tt] = pair

            # stage each JIT tile's ohT pairs two positions ahead
            if len(tile_order) > 1 and tile_order[1] >= n_pre_t:
                stage_jit(tile_order[1])
            for pos, tt in enumerate(tile_order):
                if pos + 2 < len(tile_order) and tile_order[pos + 2] >= n_pre_t:
                    stage_jit(tile_order[pos + 2])
                jit = tt >= n_pre_t
                ob4 = outp.tile([P, FB], bf16, tag="ob")
                for k in range(4):
                    g = 4 * tt + k
                    if jit:
                        rhs_ap = jit_ohs[tt][k // 2][0:NCLS, (k % 2) * 512 : (k % 2 + 1) * 512]
                    else:
                        rhs_ap = ohstash[0:NCLS, g * 512 : (g + 1) * 512]
                    op_ = psA.tile([P, 512], f32, tag="a")
                    nc.tensor.matmul(
                        out=op_[:], lhsT=meansT[:], rhs=rhs_ap, start=True, stop=True
                    )
                    rot = ((1, 0, 1, 0), (0, 1, 0, 1))[pos % 2]
                    copy_by(rot[k], ob4[:, k * 512 : (k + 1) * 512], op_[:])
                if pos == 0 or pos == len(tile_order) - 1:
                    # split first tiles' DMAs (stream starts sooner after
                    # means) and the last tile's (tail drains sooner)
                    for s in range(4):
                        nc.sync.dma_start(
                            out=out.ap()[:, (4 * tt + s) * 512 : (4 * tt + s + 1) * 512],
                            in_=ob4[:, s * 512 : (s + 1) * 512],
                        )
                else:
                    nc.sync.dma_start(
                        out=out.ap()[:, tt * FB : (tt + 1) * FB], in_=ob4[:]
                    )

    nc.compile()
    return nc


def get_module():
    if "nc" not in _CACHE:
        _CACHE["nc"] = _build_module()
    return _CACHE["nc"]


def kernel(img, gt):
    from concourse.bass_utils import run_bass_kernel_spmd

    img = np.asarray(img)
    gt = np.asarray(gt)
    B, C, H, W = img.shape
    assert (B, C, H * W) == (N_CORES, P, HW), (img.shape,)
    img2 = np.ascontiguousarray(img.reshape(B, C, H * W))
    gt2 = np.ascontiguousarray(gt.reshape(B, H * W))

    nc = get_module()
    in_maps = [{"img": img2[i], "gt": gt2[i]} for i in range(B)]
    res = run_bass_kernel_spmd(nc, in_maps, core_ids=list(range(N_CORES)))
    out = np.stack(
        [np.asarray(res.results[i]["out"]).astype(np.float32) for i in range(B)],
        axis=0,
    )
    return out.reshape(B, C, H, W)


if __name__ == "__main__":
    rng = np.random.default_rng(0)
    img = rng.standard_normal((8, 128, 256, 256), dtype=np.float32)
    gt = rng.integers(0, NCLS, size=(8, 1, 256, 256), dtype=np.int32)
    out = kernel(img=img, gt=gt)
    print("out", out.shape, out.dtype)


# revision 69
# speedup vs baseline: 1.0130x; 1.0004x over previous
"""Trainium2 Bass kernel: per-(image, channel) class-mean replacement (segment mean + gather).

Input:  img [8, 128, 256, 256] f32, gt [8, 1, 256, 256] int32 (labels in [0, 21))
Output: out[b, c, h, w] = mean over pixels p of img[b, c, p] where gt[b, p] == gt[b, h, w]

Sharding: data-parallel over batch — each of the 8 NeuronCores processes one image.

Per-core algorithm (C=128 channels on partitions, HW=65536 pixels on free axis):
  Setup:    gt -> chunk-major gtT [128pix, 512chunk] via PE transposes;
            class-major one-hot planes stash[p, c*512+gc] = (gtT[p,gc]==c),
            built in 64-chunk spans: span 0 on DVE (gates the first sums
            matmul); spans 1-7 split DVE (in-loop) / Pool (upfront).
  Phase 1:  PE-transpose img chunks; copy PSUM->SBUF with
            ->bf16 cast (DVE/Act alternating); sums matmul SWAPPED: stationary =
            imgT chunk [128px,128ch], moving = one-hot view [128px,21cls] ->
            accumulate sumsT[128ch,21cls] in PSUM (21-col outputs are nearly
            free). Counts via lhsT=onehot, rhs=ones -> cnt[21,1]. Sums matmuls
            issue two 512-px groups late (software pipelining) so the in-order
            PE queue never blocks on the copies. Phase-2 ohT pre-transposes for
            the first PRE_G groups are interleaved (Pool joins the copy
            rotation only after its span work is done).
  Means:    sumsT -> SBUF -> PE-transpose -> meansT[21,128] bf16 = sums*rcp(cnt).
  Phase 2:  out[128ch,512px] = meansT^T @ ohT[21,512] per group; copy PSUM->SBUF
            as bf16; DMA out 2048-px tiles. Output DRAM tensor is bf16 (host
            casts back to f32) — halves write bandwidth at zero added error
            since means are already bf16. Pre-transposed and JIT output tiles
            alternate 1:1; JIT ohT pairs are transposed two tiles ahead.
            NOTE: GPSIMD/Pool must never read PSUM (walrus rejects it), so all
            PSUM->SBUF copies stay on DVE/Act.
"""

import os
import sys

for _p in ("/opt/trn_rl_repo", "/root/.axon_site/_ro/trn_rl_repo"):
    if os.path.isdir(_p) and _p not in sys.path:
        sys.path.append(_p)

import numpy as np

P = 128          # channels == SBUF partitions
HW = 256 * 256   # pixels per image
NCLS = 21
CH = 128         # pixels per matmul chunk
NCH = HW // CH   # 512 chunks
FB = 2048        # pixels per DMA tile
NB = HW // FB    # 32 big tiles
NGR = HW // 512  # 128 phase-2 groups (512 px each)
PRE_G = 92       # groups whose ohT is pre-transposed during phase 1
EPS = 1e-8
N_CORES = 8

_CACHE = {}


def _build_module():
    import concourse.bacc as bacc
    import concourse.mybir as mybir
    import concourse.tile as tile
    from concourse.masks import make_identity

    f32 = mybir.dt.float32
    bf16 = mybir.dt.bfloat16
    i32 = mybir.dt.int32
    EQ = mybir.AluOpType.is_equal
    MULT = mybir.AluOpType.mult

    nc = bacc.Bacc("TRN2", target_bir_lowering=False, debug=False)
    img = nc.dram_tensor("img", [P, HW], f32, kind="ExternalInput")
    gt = nc.dram_tensor("gt", [HW], i32, kind="ExternalInput")
    out = nc.dram_tensor("out", [P, HW], bf16, kind="ExternalOutput")

    with tile.TileContext(nc) as tc:
        with (
            tc.tile_pool(name="constp", bufs=1) as constp,
            tc.tile_pool(name="imgp", bufs=5) as imgp,
            tc.tile_pool(name="rhsp", bufs=12) as rhsp,
            tc.tile_pool(name="ohsbp", bufs=2) as ohsbp,
            tc.tile_pool(name="outp", bufs=5) as outp,
            tc.tile_pool(name="psA", bufs=5, space="PSUM") as psA,
            tc.tile_pool(name="psB", bufs=1, space="PSUM") as psB,
            tc.tile_pool(name="psC", bufs=2, space="PSUM") as psC,
        ):
            # ---- constants ----
            ident32 = constp.tile([P, P], f32, tag="id32")
            make_identity(nc, ident32[:])
            ident16 = constp.tile([P, P], bf16, tag="id16")
            nc.vector.tensor_copy(out=ident16[:], in_=ident32[:])
            ones1 = constp.tile([P, 1], bf16, tag="ones1")
            nc.vector.memset(ones1[:], 1.0)

            # gt: load [32, 2048], cast f32 (Act), PE-transpose 16 blocks into
            # chunk-major gtT (block b holds chunks {16r+b}; stride-16 dest AP).
            # gt staging borrows imgp slots (same per-partition footprint).
            gtn_i = imgp.tile([32, HW // 32], i32, tag="img")
            gtn = imgp.tile([32, HW // 32], f32, tag="img")
            # gt loads FIRST on the SP queue (ahead of the img stream on the
            # serial DMA engines) in 2 pieces, casts pipelined on Act
            gt_pc = (HW // 32) // 2
            for pc in range(2):
                nc.sync.dma_start(
                    out=gtn_i[:, pc * gt_pc : (pc + 1) * gt_pc],
                    in_=gt.ap().rearrange("(p f) -> p f", p=32)[
                        :, pc * gt_pc : (pc + 1) * gt_pc
                    ],
                )
            for pc in range(2):
                nc.scalar.copy(
                    out=gtn[:, pc * gt_pc : (pc + 1) * gt_pc],
                    in_=gtn_i[:, pc * gt_pc : (pc + 1) * gt_pc],
                )
            gtT = constp.tile([P, NCH], f32, tag="gtT")
            gtTv = gtT[:].rearrange("p (r b) -> p r b", b=16)
            # all 16 [32,128]->[128,32] block transposes land in ONE psA tile,
            # then a single strided copy scatters them into chunk-major gtT —
            # avoids 16 cross-engine sem round-trips through a 2-deep pool
            gps16 = psA.tile([P, 512], f32, tag="a")
            for b in range(16):
                nc.tensor.transpose(
                    out=gps16[:, b * 32 : (b + 1) * 32],
                    in_=gtn[:, b * P : (b + 1) * P],
                    identity=ident32[0:32, 0:32],
                )
            nc.vector.tensor_copy(
                out=gtTv[:, :, :],
                in_=gps16[:].rearrange("p (b r) -> p r b", b=16),
            )

            # class-major one-hot planes: stash[p, c*NCH + gc] = (gtT[p,gc]==c)
            stash = constp.tile([P, NCLS * NCH], bf16, tag="stash")

            def issue_span(s, eng):
                for c in range(NCLS):
                    eng.tensor_scalar(
                        stash[:, c * NCH + 64 * s : c * NCH + 64 * (s + 1)],
                        gtT[:, 64 * s : 64 * (s + 1)],
                        float(c),
                        None,
                        EQ,
                    )

            # span 0 on DVE (it gates the first sums matmul). Spans 1-7 are
            # split by class: Pool halves issue upfront (Pool is idle), DVE
            # halves issue from inside the tile loop so the rhs copies are not
            # queued behind them — span s gates only tiles 4s and later.
            issue_span(0, nc.vector)

            def issue_span_half(s, eng_id):
                for c in range(NCLS):
                    if c % 2 == eng_id:
                        continue
                    eng = nc.vector if c % 2 == 0 else nc.gpsimd
                    eng.tensor_scalar(
                        stash[:, c * NCH + 64 * s : c * NCH + 64 * (s + 1)],
                        gtT[:, 64 * s : 64 * (s + 1)],
                        float(c),
                        None,
                        EQ,
                    )

            for s in range(1, 8):
                issue_span_half(s, 0)  # odd classes -> Pool, upfront
            stashv = stash[:].rearrange("p (c j) -> p c j", c=NCLS)

            def ohview(gc):
                return stashv[:, :, gc]  # [128px, 21cls]

            # pre-transposed ohT for groups [0, PRE_G)
            ohstash = constp.tile([32, PRE_G * 512], bf16, tag="ohstash")

            sums = psB.tile([P, NCLS], f32, tag="sums")
            # counts: 21 full-tile (XYZWC) reductions of the one-hot planes on
            # Pool into a [1,21] row (partition 0) — no PSUM bank, no PE work
            cntrow = constp.tile([1, NCLS], f32, tag="cntrow")

            def copy_by(eng, dst, src):
                if eng == 0:
                    nc.vector.tensor_copy(out=dst, in_=src)
                elif eng == 1:
                    nc.scalar.copy(out=dst, in_=src)
                else:
                    nc.gpsimd.tensor_copy(out=dst, in_=src)

            def pre_transpose_group(g, eng):
                ohps = psC.tile([32, 512], bf16, tag="c")
                for q in range(4):
                    nc.tensor.transpose(
                        out=ohps[0:NCLS, q * CH : (q + 1) * CH],
                        in_=ohview(g * 4 + q),
                        identity=ident16[:],
                    )
                copy_by(eng, ohstash[0:NCLS, g * 512 : (g + 1) * 512], ohps[0:NCLS, :])

            # ---- phase 1 ----
            LAG = 4
            pre_done = 0
            pending = []

            def issue_sums(g4, rhs4):
                for q in range(4):
                    gc = g4 * 4 + q
                    nc.tensor.matmul(
                        out=sums[:],
                        lhsT=rhs4[:, q * CH : (q + 1) * CH],
                        rhs=ohview(gc),
                        start=(gc == 0),
                        stop=(gc == NCH - 1),
                    )

            pre_done = 0
            for t in range(NB):
                ib = imgp.tile([P, FB], f32, tag="img")
                for h in range(2):
                    nc.sync.dma_start(
                        out=ib[:, h * 1024 : (h + 1) * 1024],
                        in_=img.ap()[:, t * FB + h * 1024 : t * FB + (h + 1) * 1024],
                    )
                for jj in range(4):
                    g4 = t * 4 + jj
                    tp4 = psA.tile([P, 512], f32, tag="a")
                    for q in range(4):
                        nc.tensor.transpose(
                            out=tp4[:, q * CH : (q + 1) * CH],
                            in_=ib[:, (jj * 4 + q) * CH : (jj * 4 + q + 1) * CH],
                            identity=ident32[:],
                        )
                    rhs4 = rhsp.tile([P, 512], bf16, tag="rhs")
                    copy_by(g4 % 2, rhs4[:], tp4[:])
                    pending.append((g4, rhs4))
                    if len(pending) > LAG:
                        issue_sums(*pending.pop(0))
                # DVE half of span t//4+1 right after this tile's copies
                if t % 4 == 0 and t // 4 + 1 < 8:
                    issue_span_half(t // 4 + 1, 1)  # even classes -> DVE
                # two phase-2 ohT pre-transposes per tile; Pool joins the
                # copy rotation only once its span building is long done
                target = min(PRE_G // 2, ((t + 1) * (PRE_G // 2)) // NB)
                while pre_done < target:
                    pre_transpose_group(2 * pre_done, pre_done % 2)
                    pre_transpose_group(2 * pre_done + 1, (pre_done + 1) % 2)
                    pre_done += 1
            while pending:
                issue_sums(*pending.pop(0))

            # counts: odd classes first (their planes finish on Pool early),
            # even classes after the in-loop DVE span halves complete
            for c in [c for c in range(NCLS) if c % 2] + [c for c in range(NCLS) if c % 2 == 0]:
                nc.gpsimd.tensor_reduce(
                    out=cntrow[0:1, c : c + 1],
                    in_=stash[:, c * NCH : (c + 1) * NCH],
                    axis=mybir.AxisListType.XYZWC,
                    op=mybir.AluOpType.add,
                )

            # ---- means: meansT[21,128] bf16 = sumsT^T * 1/(cnt+eps) ----
            cntP = psC.tile([NCLS, 1], f32, tag="c")
            nc.tensor.transpose(
                out=cntP[:], in_=cntrow[:], identity=ident32[0:1, 0:1]
            )
            cnte = constp.tile([NCLS, 1], f32, tag="cnte")
            nc.vector.tensor_scalar_add(cnte[:], cntP[:], EPS)
            rcp = constp.tile([NCLS, 1], f32, tag="rcp")
            nc.vector.reciprocal(out=rcp[:], in_=cnte[:])
            sms = constp.tile([P, NCLS], f32, tag="sms")
            nc.vector.tensor_copy(out=sms[:], in_=sums[:])
            smsP = psC.tile([NCLS, P], f32, tag="c")
            nc.tensor.transpose(out=smsP[:], in_=sms[:], identity=ident32[:])
            meansT = constp.tile([NCLS, P], bf16, tag="meansT")
            nc.vector.tensor_scalar(meansT[:], smsP[:], rcp[:, 0:1], None, MULT)

            # ---- phase 2: out[128ch, px] = meansT^T @ ohT ----
            # Pre-transposed and JIT output tiles alternate 1:1; each JIT
            # tile's two ohT pairs are transposed+copied one position ahead.
            n_pre_t = PRE_G // 4
            n_jit = NB - n_pre_t
            tile_order, pi, ji, err = [], 0, n_pre_t, 0
            for k in range(NB):
                err += n_jit
                if err >= NB and ji < NB and pi > 0:
                    tile_order.append(ji)
                    ji += 1
                    err -= NB
                else:
                    tile_order.append(pi)
                    pi += 1
            jit_ohs = {}
            jit_cnt = [0]

            def stage_jit(tt):
                pair = []
                for half in range(2):
                    ohps2 = psC.tile([32, 1024], bf16, tag="c")
                    for qq in range(8):
                        nc.tensor.transpose(
                            out=ohps2[0:NCLS, qq * CH : (qq + 1) * CH],
                            in_=ohview((4 * tt + 2 * half) * 4 + qq),
                            identity=ident16[:],
                        )
                    ohs = ohsbp.tile([32, 1024], bf16, tag="oh")
                    copy_by(jit_cnt[0] % 2, ohs[0:NCLS, :], ohps2[0:NCLS, :])
                    jit_cnt[0] += 1
                    pair.append(ohs)
                jit_ohs[tt] = pair

            # stage each JIT tile's ohT pairs two positions ahead
            if len(tile_order) > 1 and tile_order[1] >= n_pre_t:
                stage_jit(tile_order[1])
            for pos, tt in enumerate(tile_order):
                if pos + 2 < len(tile_order) and tile_order[pos + 2] >= n_pre_t:
                    stage_jit(tile_order[pos + 2])
                jit = tt >= n_pre_t
                ob4 = outp.tile([P, FB], bf16, tag="ob")
                for k in range(4):
                    g = 4 * tt + k
                    if jit:
                        rhs_ap = jit_ohs[tt][k // 2][0:NCLS, (k % 2) * 512 : (k % 2 + 1) * 512]
                    else:
                        rhs_ap = ohstash[0:NCLS, g * 512 : (g + 1) * 512]
                    op_ = psA.tile([P, 512], f32, tag="a")
                    nc.tensor.matmul(
                        out=op_[:], lhsT=meansT[:], rhs=rhs_ap, start=True, stop=True
                    )
                    rot = ((1, 0, 1, 0), (0, 1, 0, 1))[pos % 2]
                    copy_by(rot[k], ob4[:, k * 512 : (k + 1) * 512], op_[:])
                if pos == 0 or pos == len(tile_order) - 1:
                    # split first tiles' DMAs (stream starts sooner after
                    # means) and the last tile's (tail drains sooner)
                    for s in range(4):
                        nc.sync.dma_start(
                            out=out.ap()[:, (4 * tt + s) * 512 : (4 * tt + s + 1) * 512],
                            in_=ob4[:, s * 512 : (s + 1) * 512],
                        )
                else:
                    nc.sync.dma_start(
                        out=out.ap()[:, tt * FB : (tt + 1) * FB], in_=ob4[:]
                    )

    nc.compile()
    return nc


def get_module():
    if "nc" not in _CACHE:
        _CACHE["nc"] = _build_module()
    return _CACHE["nc"]


def kernel(img, gt):
    from concourse.bass_utils import run_bass_kernel_spmd

    img = np.asarray(img)
    gt = np.asarray(gt)
    B, C, H, W = img.shape
    assert (B, C, H * W) == (N_CORES, P, HW), (img.shape,)
    img2 = np.ascontiguousarray(img.reshape(B, C, H * W))
    gt2 = np.ascontiguousarray(gt.reshape(B, H * W))

    nc = get_module()
    in_maps = [{"img": img2[i], "gt": gt2[i]} for i in range(B)]
    res = run_bass_kernel_spmd(nc, in_maps, core_ids=list(range(N_CORES)))
    out = np.stack(
        [np.asarray(res.results[i]["out"]).astype(np.float32) for i in range(B)],
        axis=0,
    )
    return out.reshape(B, C, H, W)


if __name__ == "__main__":
    rng = np.random.default_rng(0)
    img = rng.standard_normal((8, 128, 256, 256), dtype=np.float32)
    gt = rng.integers(0, NCLS, size=(8, 1, 256, 256), dtype=np.int32)
    out = kernel(img=img, gt=gt)
    print("out", out.shape, out.dtype)
